# revision 33
# baseline (speedup 1.0000x reference)
"""Trainium2 Bass kernel for nn_CGRegressorAdapter (GNN message passing).

Strategy (cone-restricted):
  - The regression head only reads ONE node per graph (last_idx), so each
    layer of the 8-layer GNN stack only needs the node's influence cone:
    V_4={v} at the top, growing by in-neighborhoods down to V_{-1} (~1400
    nodes max) at the embed layer.  Host prep computes nested cone
    orderings (V_{k+1} is a prefix of V_k) and compacted adjacency slices
    M_l = A[V_{l-2}, V_{l-1}] (edge counts, exact in bf16).
  - Data-parallel over B=32 graphs: 8 cores x 4 slots.  Graphs are sorted
    by cone cost; slot j holds ranks [8j, 8j+8) and is sized to that
    quartile's EXACT per-level maxes (no 128-padding on free axes; the
    contraction runs 128-row chunks with a partial last chunk), so the
    small top layers cost almost nothing.
  - Adjacency slices ship as per-slot fp8-e4m3 blobs (edge counts <=16
    are exact) upcast to bf16 in-flight by SWDGE casting DMAs; embed
    inputs for all slots ship as one [40, sum Pm1] bf16 pack (embW rows
    >=40 are zero, so the matmul contracts 40 partitions); weights ship
    as two packed tiles.  DMA priority: embed weights, embed inputs, GNN
    weights, head weights, M blobs smallest slot first (big slots split
    so their L1 can start on the first half).
  - Per slot: embed (bf16 hi/lo one-hot matmul, f32-exact), 4 base + 4
    adapter GraphConvs, all matmuls single-bf16 (states bf16, weights
    bf16), f32 PSUM accumulate.  m-chunks are batched 4-at-a-time in one
    [128,512] PSUM tile and cast with a single DVE/ACT copy (alternating
    engines).  Measured end-to-end rel err 8.4e-3 vs the 2e-2 gate.
  - The four slot streams are emitted in a skewed staircase (stream i
    runs i stages behind) so layer-boundary ACT waits hide under other
    slots' matmuls and PSUM agg buffers are never oversubscribed; within
    a stream, base layer i+1 is emitted before adapter layer i (they are
    independent) to shorten the drained-tail critical chain.
  - Nested prefix ordering makes the self path a plain prefix slice and
    the final extraction column 0.  Regression head (relu-free layer
    pairs constant-folded on host) on-chip in f32.
"""
import numpy as np
import ml_dtypes

import concourse.bass as bass
import concourse.mybir as mybir
from concourse import bacc
from concourse.bass import ts
from concourse.bass_utils import run_bass_kernel_spmd
from concourse.tile import TileContext

BF16 = ml_dtypes.bfloat16
FP8 = ml_dtypes.float8_e4m3
F32 = np.float32

B, N, E, H, L, VOCAB = 32, 2048, 8192, 128, 4, 32
N_CORES = 8
NG = B // N_CORES          # graphs (slots) per core
dt = mybir.dt
Alu = mybir.AluOpType
Act = mybir.ActivationFunctionType

# bias column indices in the packed bias tile
BCOL_BASE = 0      # 0..3  base_b
BCOL_ADAPT = 4     # 4..7  adapt_b
BCOL_HB1 = 8
BCOL_HMID = 9      # 9..11
BCOL_HB5 = 12
NBCOL = 16


def _ceil128(x):
    return max(128, (int(x) + 127) // 128 * 128)


def _chunks(n):
    """[(col_off, rows)] covering n in 128-row chunks, last may be partial."""
    return [(j * 128, min(128, n - j * 128)) for j in range((n + 127) // 128)]


def _spans(width, maxw=512):
    out = []
    off = 0
    while off < width:
        w = min(maxw, width - off)
        out.append((off, w))
        off += w
    return out


def _blob_layout(sizes):
    """Free-axis offsets of the per-slot bf16 blob [128, W].
    Sections: erhs [128, Pm1], then M_l as [128, (pin/128)*pout] l=1..5."""
    Pm1, P0, P1, P2, P3 = sizes
    P4 = 1
    dims = [(Pm1, P0), (P0, P1), (P1, P2), (P2, P3), (P3, P4)]
    lay = {}
    off = 0
    for l, (pin, pout) in enumerate(dims):
        w = len(_chunks(pin)) * pout
        lay[f"m{l + 1}"] = (off, w)
        off += w
    lay["_total"] = off
    lay["_dims"] = dims
    return lay


DMA_ORDER = (3, 2, 1, 0)
MP_BUFS = 6
PSUM_AGG_BUFS = 2
PSUM_M_BUFS = 4
SKEW_ORDER = (3, 2, 1, 0)


def _build_program(slot_sizes, reps=1):
    """slot_sizes: tuple of 4 tuples (Pm1, P0, P1, P2, P3) padded sizes.
    reps>1 repeats the whole body serially (timing: slope removes
    dispatch overhead)."""
    nc = bacc.Bacc("TRN2", target_bir_lowering=False, debug=False,
                   num_devices=N_CORES)
    f32, bf16 = dt.float32, dt.bfloat16
    P4 = 1
    lays = [_blob_layout(s) for s in slot_sizes]

    # all weights packed into two tiles: bf16 (embed + GNN) and f32 (head)
    WB = 2 * H + L * 6 * H          # embw hi/lo + per layer bwn,bws,awn2,aws2
    WF = 3 * H + 1 + NBCOL          # hwa(2H) + hwb(H) + hw5(1) + biases
    wb_d = nc.declare_dram_parameter("wpack_bf", [128, WB], bf16, isOutput=False)
    wf_d = nc.declare_dram_parameter("wpack_f32", [128, WF], f32, isOutput=False)
    EP = sum(sz[0] for sz in slot_sizes)      # all slots' erhs, 40 rows
    ep_d = nc.declare_dram_parameter("epack", [40, EP], bf16, isOutput=False)
    eoffs = [sum(sz[0] for sz in slot_sizes[:s]) for s in range(NG)]
    fp8 = dt.float8e4
    blob_d = [nc.declare_dram_parameter(f"blob{s}", [128, lays[s]["_total"]],
                                        fp8, isOutput=False)
              for s in range(NG)]
    y_d = nc.declare_dram_parameter("y", [1, NG], f32, isOutput=True)

    with TileContext(nc) as tc:
        with (
            tc.tile_pool(name="const", bufs=1) as const,
            tc.tile_pool(name="state", bufs=1) as state,
            tc.tile_pool(name="mp", bufs=MP_BUFS) as mp,
            tc.tile_pool(name="psum_agg", bufs=PSUM_AGG_BUFS, space="PSUM") as psum_agg,
            tc.tile_pool(name="psum_m", bufs=PSUM_M_BUFS, space="PSUM") as psum_m,
        ):
            # ---- all input DMAs issued up front (prefetch) ----
            blob_t = [None] * NG
            ep_holder = [None]

            wb_t = const.tile([128, WB], bf16)
            wf_t = const.tile([128, WF], f32)
            consts_loaded = [False]

            def load_blobs():
                # DMA priority: embed weights, embed inputs, GNN weights,
                # head weights, then M blobs smallest slot first
                if not consts_loaded[0]:
                    nc.sync.dma_start(wb_t[:, :2 * H], wb_d[:, :2 * H])
                ep_holder[0] = state.tile([40, EP], bf16, tag="epack",
                                          name="epack")
                nc.sync.dma_start(ep_holder[0][:], ep_d[:])
                if not consts_loaded[0]:
                    nc.sync.dma_start(wb_t[:, 2 * H:], wb_d[:, 2 * H:])
                    nc.sync.dma_start(wf_t[:], wf_d[:])
                    consts_loaded[0] = True
                for s in DMA_ORDER:
                    blob_t[s] = state.tile([128, lays[s]["_total"]], bf16,
                                           tag=f"blob{s}", name=f"blob{s}")
                    half = (lays[s]["m1"][1] // 2 // 128) * 128
                    # SWDGE casting DMA: fp8 in HBM (counts are exact),
                    # bf16 in SBUF — halves the dominant DMA traffic
                    if half == 0:
                        nc.gpsimd.dma_start(blob_t[s][:], blob_d[s][:])
                    else:
                        # split so the slot's L1 can start on the first half
                        nc.gpsimd.dma_start(blob_t[s][:, :half],
                                            blob_d[s][:, :half])
                        nc.gpsimd.dma_start(blob_t[s][:, half:],
                                            blob_d[s][:, half:])
            embw_hi = wb_t[:, 0:H]
            embw_lo = wb_t[:, H:2 * H]
            bwn_t, bws_t, awn_t, aws_t = [], [], [], []
            for i in range(L):
                o = 2 * H + i * 6 * H
                bwn_t.append(wb_t[:, o:o + H])
                bws_t.append(wb_t[:, o + H:o + 2 * H])
                awn_t.append((wb_t[:, o + 2 * H:o + 3 * H],
                              wb_t[:, o + 3 * H:o + 4 * H]))
                aws_t.append((wb_t[:, o + 4 * H:o + 5 * H],
                              wb_t[:, o + 5 * H:o + 6 * H]))
            hwa0 = wf_t[:, 0:H]
            hwa1 = wf_t[:, H:2 * H]
            hwb = wf_t[:, 2 * H:3 * H]
            hw5 = wf_t[:, 3 * H:3 * H + 1]
            BOFF = 3 * H + 1

            def bias_ap(col):
                return wf_t[:, BOFF + col:BOFF + col + 1]

            gbT = state.tile([128, NG], f32, tag="gb")
            gaT = state.tile([128, NG], f32, tag="ga")

            # per-span PSUM agg tiles are fixed [128,512] and reused by tag
            def get_aggs(width):
                return [(psum_agg.tile([128, 512], f32, tag=f"agg{i % 2}",
                                       name=f"agg{i % 2}"), off, w)
                        for i, (off, w) in enumerate(_spans(width))]

            def gconv(blob, moff, nbr_srcs, self_srcs, p_in, p_out, bias_col,
                      out_tile):
                """nbr_srcs: list of (stateT [128,p_in] bf16, [W_hi, W_lo]
                rhs aps).  self_srcs: list of (stateT, [Wself hi/lo lhsT
                aps]).  blob[:, moff+j*p_out :] holds the bf16 count slice
                for chunk j."""
                chks = _chunks(p_in)       # [(col_off, rows)], exact sizes
                nchunks = len(chks)
                aggs = get_aggs(p_out)
                nterm = sum(len(ws) for _, ws in nbr_srcs)
                GW = 4                     # m chunks per grouped cast
                groups = [list(range(g, min(g + GW, nchunks)))
                          for g in range(0, nchunks, GW)]

                def emit_group(gi):
                    grp = groups[gi]
                    pm = psum_m.tile([128, 512], f32, tag="pm")
                    for jj, j in enumerate(grp):
                        co, rj = chks[j]
                        k = 0
                        for src, ws in nbr_srcs:
                            for w in ws:
                                nc.tensor.matmul(pm[:rj, jj * 128:jj * 128 + 128],
                                                 src[:, co:co + rj], w,
                                                 start=(k == 0),
                                                 stop=(k == nterm - 1))
                                k += 1
                    wd_g = len(grp) * 128
                    mhi = mp.tile([128, 512], bf16, tag="mhi")
                    if gi % 2 == 0:
                        nc.vector.tensor_copy(out=mhi[:, :wd_g], in_=pm[:, :wd_g])
                    else:
                        nc.scalar.copy(mhi[:, :wd_g], pm[:, :wd_g])
                    return mhi

                gq = [emit_group(0)]
                # self path: bf16 weights against bf16 state
                k = 0
                for src, ws in self_srcs:
                    for w in ws:
                        for a, off, wd in aggs:
                            nc.tensor.matmul(a[:, :wd], w, src[:, off:off + wd],
                                             start=(k == 0), stop=False)
                        k += 1
                for gi, grp in enumerate(groups):
                    mhi = gq.pop(0)
                    if gi + 1 < len(groups):
                        gq.append(emit_group(gi + 1))
                    for jj, j in enumerate(grp):
                        rj = chks[j][1]
                        base = moff + j * p_out
                        for a, off, wd in aggs:
                            nc.tensor.matmul(a[:, :wd],
                                             mhi[:rj, jj * 128:jj * 128 + 128],
                                             blob[:rj, base + off:base + off + wd],
                                             start=False,
                                             stop=(j == nchunks - 1))
                for a, off, wd in aggs:
                    nc.scalar.activation(out_tile[:, off:off + wd],
                                         a[:, :wd], Act.Relu,
                                         bias=bias_ap(bias_col))

            def slot_stages(s):
                """Emission closures for one slot: [embed, base1, adapt1,
                base2, ...].  Two slots are interleaved stage-by-stage so
                each layer-boundary ACT wait is hidden under the other
                slot's matmuls."""
                Pm1, P0, P1, P2, P3 = slot_sizes[s]
                lay = lays[s]
                blob = blob_t[s]
                psz = [P0, P1, P2, P3, P4]
                xT = state.tile([128, Pm1], bf16, tag=f"x{s}", name=f"x{s}")
                lat = [xT] + [state.tile([128, psz[k]], bf16, tag=f"lat{k+1}_{s}",
                                         name=f"lat{k+1}_{s}")
                              for k in range(L)]
                currs = [xT] + [state.tile([128, psz[k + 1]], bf16,
                                           tag=f"curr{k+1}_{s}",
                                           name=f"curr{k+1}_{s}")
                                for k in range(L)]
                pins = [Pm1, P0, P1, P2]
                stages = []

                def embed_stage():
                    eoff = eoffs[s]
                    ept = ep_holder[0]
                    for i_sp, (a, off, wd) in enumerate(get_aggs(Pm1)):
                        nc.tensor.matmul(a[:, :wd], embw_hi[:40, :],
                                         ept[:, eoff + off:eoff + off + wd],
                                         start=True, stop=False)
                        nc.tensor.matmul(a[:, :wd], embw_lo[:40, :],
                                         ept[:, eoff + off:eoff + off + wd],
                                         start=False, stop=True)
                        if i_sp % 2 == 0:
                            nc.vector.tensor_copy(out=xT[:, off:off + wd],
                                                  in_=a[:, :wd])
                        else:
                            nc.scalar.copy(xT[:, off:off + wd], a[:, :wd])
                stages.append(embed_stage)

                def base_stage(i):
                    def run():
                        gconv(blob, lay[f"m{i+1}"][0],
                              nbr_srcs=[(lat[i], [bwn_t[i]])],
                              self_srcs=[(lat[i], [bws_t[i]])],
                              p_in=pins[i], p_out=psz[i],
                              bias_col=BCOL_BASE + i, out_tile=lat[i + 1])
                    return run

                def adapt_stage(i):
                    def run():
                        gconv(blob, lay[f"m{i+2}"][0],
                              nbr_srcs=[(lat[i + 1], [awn_t[i][0]]),
                                        (currs[i], [awn_t[i][1]])],
                              self_srcs=[(lat[i + 1], [aws_t[i][0]]),
                                         (currs[i], [aws_t[i][1]])],
                              p_in=psz[i], p_out=psz[i + 1],
                              bias_col=BCOL_ADAPT + i, out_tile=currs[i + 1])
                        if i == L - 1:
                            nc.vector.tensor_copy(out=gbT[:, s:s + 1],
                                                  in_=lat[L][:, 0:1])
                            nc.vector.tensor_copy(out=gaT[:, s:s + 1],
                                                  in_=currs[L][:, 0:1])
                    return run

                # base_{i+1} ahead of adapt_i: they are independent, so in
                # the drained tail the base chain advances while the adapter
                # fills its ACT waits (critical depth ~6 instead of 8)
                stages.append(base_stage(0))
                for i in range(L - 1):
                    stages.append(base_stage(i + 1))
                    stages.append(adapt_stage(i))
                stages.append(adapt_stage(L - 1))
                return stages


            # ---- regression head (all slots at once) ----
            def whole_pass():
                load_blobs()
                streams = [slot_stages(ss) for ss in SKEW_ORDER]
                nst = len(streams[0])
                for r in range(nst + len(streams) - 1):
                    for i, stream in enumerate(streams):
                        k = r - i
                        if 0 <= k < nst:
                            stream[k]()
                emit_head()

            def head_mm(lhsT, rhs, bias_col, func):
                pm = psum_m.tile([128, 128], f32, tag="pm")
                nc.tensor.matmul(pm[:, :NG], lhsT, rhs, start=True, stop=True)
                out = state.tile([128, NG], f32, tag="hy")
                nc.scalar.activation(out[:], pm[:, :NG], func,
                                     bias=bias_ap(bias_col))
                return out

            def emit_head():
                # head with relu-free pairs constant-folded on host:
                # y = ((relu(g@Wa+ba))@Wb+bb -> relu) @ hW5 + hb5
                pm = psum_m.tile([128, 128], f32, tag="pm")
                nc.tensor.matmul(pm[:, :NG], hwa0, gbT[:],
                                 start=True, stop=False)
                nc.tensor.matmul(pm[:, :NG], hwa1, gaT[:],
                                 start=False, stop=True)
                y1 = state.tile([128, NG], f32, tag="hy")
                nc.scalar.activation(y1[:], pm[:, :NG], Act.Relu,
                                     bias=bias_ap(BCOL_HB1))
                y2 = head_mm(hwb, y1[:], BCOL_HMID + 0, Act.Relu)
                pm5 = psum_m.tile([128, 128], f32, tag="pm")
                nc.tensor.matmul(pm5[:1, :NG], hw5, y2[:],
                                 start=True, stop=True)
                yout = state.tile([1, NG], f32, tag="yout")
                nc.scalar.activation(yout[:], pm5[:1, :NG], Act.Identity,
                                     bias=bias_ap(BCOL_HB5)[:1])
                nc.sync.dma_start(y_d[:], yout[:])

            for _rep in range(reps):
                whole_pass()

    nc.compile()
    return nc


_NC_CACHE = {}
_LAST = {}


def _get_program(reps=1):
    key = (_LAST["slot_sizes"], reps)
    if key not in _NC_CACHE:
        _NC_CACHE[key] = _build_program(_LAST["slot_sizes"], reps=reps)
    return _NC_CACHE[key]


def _cones(edge, last_idx):
    """Nested cone ordering per graph.  Returns (order, sizes[n4..nm1])."""
    out = []
    for g in range(B):
        src, dst = edge[g, 0], edge[g, 1]
        order = [int(last_idx[g])]
        inset = np.zeros(N, bool)
        inset[order[0]] = True
        sizes = [1]
        for _ in range(5):
            new = np.unique(src[inset[dst]])
            new = new[~inset[new]]
            order.extend(new.tolist())
            inset[new] = True
            sizes.append(len(order))
        out.append((np.asarray(order), sizes))
    return out


def _split_hilo(a):
    hi = a.astype(BF16)
    lo = (a - hi.astype(F32)).astype(BF16)
    return hi, lo


def _prep_inputs(inputs):
    """Host-side cone construction + sharding.  Returns list of in_maps."""
    inds = np.asarray(inputs["regular_node_inds"]).astype(np.int64)
    shapes = np.asarray(inputs["regular_node_shapes"], dtype=F32)
    edge = np.asarray(inputs["edge_index"]).astype(np.int64)
    last_idx = np.asarray(inputs["last_idx"]).astype(np.int64)

    cones = _cones(edge, last_idx)
    # sort graphs by cost; slot j <- ranks [8j, 8j+8), core c <- rank 8j+c
    cost = np.array([c[1][5] + c[1][4] for c in cones])
    ranks = np.argsort(-cost, kind="stable")
    assign = ranks.reshape(NG, N_CORES)          # [slot, core] -> graph id
    slot_sizes = []
    for s in range(NG):
        gs = assign[s]
        mx = [max(cones[g][1][k] for g in gs) for k in range(6)]
        # sizes[k] = |V_{4-k}|; exact per-level maxes (Pm1,P0,P1,P2,P3)
        slot_sizes.append(tuple(int(mx[5 - l]) for l in range(5)))
    slot_sizes = tuple(slot_sizes)
    _LAST["slot_sizes"] = slot_sizes
    _LAST["assign"] = assign
    lays = [_blob_layout(s) for s in slot_sizes]

    # embed weights, hi/lo bf16 pair (exact): rows 0..31 table, 32..35 and
    # 36..39 shape_w (paired against shapes_hi / shapes_lo blob rows)
    embed_w = np.zeros((128, H), dtype=F32)
    embed_w[:VOCAB] = np.asarray(inputs["embed_table"], dtype=F32)
    embed_w[VOCAB:VOCAB + 4] = np.asarray(inputs["shape_w"], dtype=F32)
    embed_w[VOCAB + 4:VOCAB + 8] = np.asarray(inputs["shape_w"], dtype=F32)
    ehi, elo = _split_hilo(embed_w)
    # the shape_w rows must stay IDENTICAL in both copies within each of
    # hi/lo (they are), pairing: x = oh@(thi+tlo) + (shi+slo)@(swhi+swlo)
    embed_w2 = np.stack([ehi, elo], axis=1)     # [128, 2, H]

    bws2 = np.asarray(inputs["base_Wself"], dtype=F32).astype(BF16)
    bwn2 = np.asarray(inputs["base_Wnbr"], dtype=F32).astype(BF16)
    aws = np.asarray(inputs["adapt_Wself"], dtype=F32).reshape(L, 2, H, H)
    awn = np.asarray(inputs["adapt_Wnbr"], dtype=F32).reshape(L, 2, H, H)
    aws2 = np.ascontiguousarray(aws.transpose(0, 2, 1, 3)).astype(BF16)
    awn2 = np.ascontiguousarray(awn.transpose(0, 2, 1, 3)).astype(BF16)
    hW1 = np.asarray(inputs["hW1"], np.float64)
    hb1 = np.asarray(inputs["hb1"], np.float64)
    hWm = np.asarray(inputs["hWmid"], np.float64)
    hbm = np.asarray(inputs["hbmid"], np.float64)
    Wa = hW1 @ hWm[0]                       # [2H, H]
    ba = hb1 @ hWm[0] + hbm[0]
    Wb = hWm[1] @ hWm[2]                    # [H, H]
    bb = hbm[1] @ hWm[2] + hbm[2]
    hw1 = np.ascontiguousarray(
        Wa.astype(F32).reshape(2, H, H).transpose(1, 0, 2))

    biases = np.zeros((H, NBCOL), dtype=F32)
    biases[:, BCOL_BASE:BCOL_BASE + L] = np.asarray(inputs["base_b"], dtype=F32).T
    biases[:, BCOL_ADAPT:BCOL_ADAPT + L] = np.asarray(inputs["adapt_b"], dtype=F32).T
    biases[:, BCOL_HB1] = ba.astype(F32)
    biases[:, BCOL_HMID] = bb.astype(F32)
    biases[0, BCOL_HB5] = np.asarray(inputs["hb5"], dtype=F32)[0]

    WB = 2 * H + L * 6 * H
    WF = 3 * H + 1 + NBCOL
    wpack_bf = np.zeros((128, WB), dtype=BF16)
    wpack_bf[:, 0:H] = embed_w2[:, 0, :]
    wpack_bf[:, H:2 * H] = embed_w2[:, 1, :]
    for i in range(L):
        o = 2 * H + i * 6 * H
        wpack_bf[:, o:o + H] = bwn2[i]
        wpack_bf[:, o + H:o + 2 * H] = bws2[i]
        wpack_bf[:, o + 2 * H:o + 3 * H] = awn2[i][:, 0, :]
        wpack_bf[:, o + 3 * H:o + 4 * H] = awn2[i][:, 1, :]
        wpack_bf[:, o + 4 * H:o + 5 * H] = aws2[i][:, 0, :]
        wpack_bf[:, o + 5 * H:o + 6 * H] = aws2[i][:, 1, :]
    wpack_f32 = np.zeros((128, WF), dtype=F32)
    wpack_f32[:, 0:H] = Wa.astype(F32)[:H, :]
    wpack_f32[:, H:2 * H] = Wa.astype(F32)[H:, :]
    wpack_f32[:, 2 * H:3 * H] = Wb.astype(F32)
    wpack_f32[:, 3 * H:3 * H + 1] = np.asarray(inputs["hW5"], dtype=F32)
    wpack_f32[:, 3 * H + 1:] = biases
    shared = {"wpack_bf": wpack_bf, "wpack_f32": wpack_f32}
    in_maps = [dict(shared) for _ in range(N_CORES)]
    EP = sum(sz[0] for sz in slot_sizes)
    epack = [np.zeros((40, EP), dtype=BF16) for _ in range(N_CORES)]
    for s in range(NG):
        Pm1, P0, P1, P2, P3 = slot_sizes[s]
        lay = lays[s]
        for c in range(N_CORES):
            g = assign[s, c]
            order, sizes = cones[g]
            n = len(order)
            pos = np.full(N, -1, np.int64)
            pos[order] = np.arange(n)
            src, dst = edge[g, 0], edge[g, 1]
            ps, pd = pos[src], pos[dst]
            blob = np.zeros((128, lay["_total"]), dtype=FP8)
            # erhs: one-hot rows 0..31, shapes hi rows 32..35, lo rows 36..39
            eoff = sum(sz[0] for sz in slot_sizes[:s])
            erhs = np.zeros((40, Pm1), dtype=F32)
            erhs[inds[g][order], np.arange(n)] = 1.0
            shi, slo = _split_hilo(shapes[g][order].T)
            epack[c][:, eoff:eoff + Pm1] = erhs.astype(BF16)
            epack[c][VOCAB:VOCAB + 4, eoff:eoff + n] = shi[:, :n]
            epack[c][VOCAB + 4:VOCAB + 8, eoff:eoff + n] = slo[:, :n]
            for l, (pin, pout) in enumerate(lay["_dims"]):
                ncols = sizes[4 - l]   # |V_{l-1}|
                rceil = ((pin + 127) // 128) * 128
                M = np.zeros((rceil, pout), dtype=F32)
                mask = (pd >= 0) & (pd < ncols)
                np.add.at(M, (ps[mask], pd[mask]), 1.0)
                moff = lay[f"m{l + 1}"][0]
                # chunk-major on the free axis, stride pout, exact widths
                assert M.max() <= 16, "edge multiplicity exceeds fp8-exact range"
                Mt = M.astype(FP8).reshape(rceil // 128, 128, pout)
                blob[:, moff:moff + (rceil // 128) * pout] = (
                    Mt.transpose(1, 0, 2).reshape(128, -1))
            in_maps[c][f"blob{s}"] = blob
    for c in range(N_CORES):
        in_maps[c]["epack"] = epack[c]
    return in_maps


def kernel(**inputs) -> np.ndarray:
    in_maps = _prep_inputs(inputs)
    nc = _get_program()
    assign = _LAST["assign"]
    # first dispatch after a fresh compile has produced garbage before
    # (axon staging race); run twice and keep the steady-state result
    run_bass_kernel_spmd(nc, in_maps, core_ids=list(range(N_CORES)))
    res = run_bass_kernel_spmd(nc, in_maps, core_ids=list(range(N_CORES)))
    out = np.zeros((B, 1), dtype=F32)
    for c in range(N_CORES):
        yc = np.asarray(res.results[c]["y"]).reshape(NG)
        for s in range(NG):
            out[assign[s, c], 0] = yc[s]
    return out


# revision 36
# speedup vs baseline: 1.0322x; 1.0322x over previous
"""Trainium2 Bass kernel for nn_CGRegressorAdapter (GNN message passing).

Strategy (cone-restricted):
  - The regression head only reads ONE node per graph (last_idx), so each
    layer of the 8-layer GNN stack only needs the node's influence cone:
    V_4={v} at the top, growing by in-neighborhoods down to V_{-1} (~1400
    nodes max) at the embed layer.  Host prep computes nested cone
    orderings (V_{k+1} is a prefix of V_k) and compacted adjacency slices
    M_l = A[V_{l-2}, V_{l-1}] (edge counts, exact in bf16).
  - Data-parallel over B=32 graphs: 8 cores x 4 slots.  Graphs are sorted
    by cone cost; slot j holds ranks [8j, 8j+8) and is sized to that
    quartile's EXACT per-level maxes (no 128-padding on free axes; the
    contraction runs 128-row chunks with a partial last chunk), so the
    small top layers cost almost nothing.
  - Adjacency slices ship as per-slot fp8-e4m3 blobs (edge counts <=16
    are exact) upcast to bf16 in-flight by SWDGE casting DMAs; embed
    inputs for all slots ship as one [40, sum Pm1] bf16 pack (embW rows
    >=40 are zero, so the matmul contracts 40 partitions); weights ship
    as two packed tiles.  DMA priority: embed weights, embed inputs, GNN
    weights, head weights, M blobs smallest slot first (big slots split
    so their L1 can start on the first half).
  - Per slot: embed (bf16 hi/lo one-hot matmul, f32-exact), 4 base + 4
    adapter GraphConvs, all matmuls single-bf16 (states bf16, weights
    bf16), f32 PSUM accumulate.  m-chunks are batched 4-at-a-time in one
    [128,512] PSUM tile and cast with a single DVE/ACT copy (alternating
    engines).  Measured end-to-end rel err 8.4e-3 vs the 2e-2 gate.
  - The four slot streams are emitted in a skewed staircase (stream i
    runs i stages behind) so layer-boundary ACT waits hide under other
    slots' matmuls and PSUM agg buffers are never oversubscribed; within
    a stream, base layer i+1 is emitted before adapter layer i (they are
    independent) to shorten the drained-tail critical chain.
  - Nested prefix ordering makes the self path a plain prefix slice and
    the final extraction column 0.  Regression head (relu-free layer
    pairs constant-folded on host) on-chip in f32.
"""
import numpy as np
import ml_dtypes

import concourse.bass as bass
import concourse.mybir as mybir
from concourse import bacc
from concourse.bass import ts
from concourse.bass_utils import run_bass_kernel_spmd
from concourse.tile import TileContext

BF16 = ml_dtypes.bfloat16
FP8 = ml_dtypes.float8_e4m3
F32 = np.float32

B, N, E, H, L, VOCAB = 32, 2048, 8192, 128, 4, 32
N_CORES = 8
NG = B // N_CORES          # graphs (slots) per core
dt = mybir.dt
Alu = mybir.AluOpType
Act = mybir.ActivationFunctionType

# bias column indices in the packed bias tile
BCOL_BASE = 0      # 0..3  base_b
BCOL_ADAPT = 4     # 4..7  adapt_b
BCOL_HB1 = 8
BCOL_HMID = 9      # 9..11
BCOL_HB5 = 12
NBCOL = 16


def _ceil128(x):
    return max(128, (int(x) + 127) // 128 * 128)


def _chunks(n):
    """[(col_off, rows)] covering n in 128-row chunks, last may be partial."""
    return [(j * 128, min(128, n - j * 128)) for j in range((n + 127) // 128)]


def _spans(width, maxw=512):
    out = []
    off = 0
    while off < width:
        w = min(maxw, width - off)
        out.append((off, w))
        off += w
    return out


def _blob_layout(sizes):
    """Free-axis offsets of the per-slot bf16 blob [128, W].
    Sections: erhs [128, Pm1], then M_l as [128, (pin/128)*pout] l=1..5."""
    Pm1, P0, P1, P2, P3 = sizes
    P4 = 1
    dims = [(Pm1, P0), (P0, P1), (P1, P2), (P2, P3), (P3, P4)]
    lay = {}
    off = 0
    for l, (pin, pout) in enumerate(dims):
        w = len(_chunks(pin)) * pout
        lay[f"m{l + 1}"] = (off, w)
        off += w
    lay["_total"] = off
    lay["_dims"] = dims
    return lay


DMA_ORDER = (3, 2, 1, 0)
MP_BUFS = 6
PSUM_AGG_BUFS = 2
PSUM_M_BUFS = 4
SKEW_ORDER = (3, 2, 1, 0)


def _build_program(slot_sizes, reps=1):
    """slot_sizes: tuple of 4 tuples (Pm1, P0, P1, P2, P3) padded sizes.
    reps>1 repeats the whole body serially (timing: slope removes
    dispatch overhead)."""
    nc = bacc.Bacc("TRN2", target_bir_lowering=False, debug=False,
                   num_devices=N_CORES)
    f32, bf16 = dt.float32, dt.bfloat16
    P4 = 1
    lays = [_blob_layout(s) for s in slot_sizes]

    # all weights packed into two tiles: bf16 (embed + GNN) and f32 (head)
    WB = 2 * H + L * 6 * H          # embw hi/lo + per layer bwn,bws,awn2,aws2
    WF = 3 * H + 1 + NBCOL          # hwa(2H) + hwb(H) + hw5(1) + biases
    wb_d = nc.declare_dram_parameter("wpack_bf", [128, WB], bf16, isOutput=False)
    wf_d = nc.declare_dram_parameter("wpack_f32", [128, WF], f32, isOutput=False)
    EP = sum(sz[0] for sz in slot_sizes)      # all slots' erhs, 40 rows
    ep_d = nc.declare_dram_parameter("epack", [40, EP], bf16, isOutput=False)
    eoffs = [sum(sz[0] for sz in slot_sizes[:s]) for s in range(NG)]
    fp8 = dt.float8e4
    blob_d = [nc.declare_dram_parameter(f"blob{s}", [128, lays[s]["_total"]],
                                        fp8, isOutput=False)
              for s in range(NG)]
    y_d = nc.declare_dram_parameter("y", [1, NG], f32, isOutput=True)

    with TileContext(nc) as tc:
        with (
            tc.tile_pool(name="const", bufs=1) as const,
            tc.tile_pool(name="state", bufs=1) as state,
            tc.tile_pool(name="mp", bufs=MP_BUFS) as mp,
            tc.tile_pool(name="psum_agg", bufs=PSUM_AGG_BUFS, space="PSUM") as psum_agg,
            tc.tile_pool(name="psum_m", bufs=PSUM_M_BUFS, space="PSUM") as psum_m,
        ):
            # ---- all input DMAs issued up front (prefetch) ----
            blob_t = [None] * NG
            ep_holder = [None]

            wb_t = const.tile([128, WB], bf16)
            wf_t = const.tile([128, WF], f32)
            consts_loaded = [False]

            def load_blobs():
                # DMA priority: embed weights, embed inputs, GNN weights,
                # head weights, then M blobs smallest slot first
                if not consts_loaded[0]:
                    nc.sync.dma_start(wb_t[:, :2 * H], wb_d[:, :2 * H])
                ep_holder[0] = state.tile([40, EP], bf16, tag="epack",
                                          name="epack")
                nc.sync.dma_start(ep_holder[0][:], ep_d[:])
                if not consts_loaded[0]:
                    nc.sync.dma_start(wb_t[:, 2 * H:], wb_d[:, 2 * H:])
                    nc.sync.dma_start(wf_t[:], wf_d[:])
                    consts_loaded[0] = True
                for s in DMA_ORDER:
                    blob_t[s] = state.tile([128, lays[s]["_total"]], bf16,
                                           tag=f"blob{s}", name=f"blob{s}")
                    half = (lays[s]["m1"][1] // 2 // 128) * 128
                    # SWDGE casting DMA: fp8 in HBM (counts are exact),
                    # bf16 in SBUF — halves the dominant DMA traffic
                    if half == 0:
                        nc.gpsimd.dma_start(blob_t[s][:], blob_d[s][:])
                    else:
                        # split so the slot's L1 can start on the first half
                        nc.gpsimd.dma_start(blob_t[s][:, :half],
                                            blob_d[s][:, :half])
                        nc.gpsimd.dma_start(blob_t[s][:, half:],
                                            blob_d[s][:, half:])
            embw_hi = wb_t[:, 0:H]
            embw_lo = wb_t[:, H:2 * H]
            bwn_t, bws_t, awn_t, aws_t = [], [], [], []
            for i in range(L):
                o = 2 * H + i * 6 * H
                bwn_t.append(wb_t[:, o:o + H])
                bws_t.append(wb_t[:, o + H:o + 2 * H])
                awn_t.append((wb_t[:, o + 2 * H:o + 3 * H],
                              wb_t[:, o + 3 * H:o + 4 * H]))
                aws_t.append((wb_t[:, o + 4 * H:o + 5 * H],
                              wb_t[:, o + 5 * H:o + 6 * H]))
            hwa0 = wf_t[:, 0:H]
            hwa1 = wf_t[:, H:2 * H]
            hwb = wf_t[:, 2 * H:3 * H]
            hw5 = wf_t[:, 3 * H:3 * H + 1]
            BOFF = 3 * H + 1

            def bias_ap(col):
                return wf_t[:, BOFF + col:BOFF + col + 1]

            gbT = state.tile([128, NG], f32, tag="gb")
            gaT = state.tile([128, NG], f32, tag="ga")

            # per-span PSUM agg tiles are fixed [128,512] and reused by tag
            def get_aggs(width):
                return [(psum_agg.tile([128, 512], f32, tag=f"agg{i % 2}",
                                       name=f"agg{i % 2}"), off, w)
                        for i, (off, w) in enumerate(_spans(width))]

            def gconv(blob, moff, nbr_srcs, self_srcs, p_in, p_out, bias_col,
                      out_tile):
                """nbr_srcs: list of (stateT [128,p_in] bf16, [W_hi, W_lo]
                rhs aps).  self_srcs: list of (stateT, [Wself hi/lo lhsT
                aps]).  blob[:, moff+j*p_out :] holds the bf16 count slice
                for chunk j."""
                chks = _chunks(p_in)       # [(col_off, rows)], exact sizes
                nchunks = len(chks)
                aggs = get_aggs(p_out)
                nterm = sum(len(ws) for _, ws in nbr_srcs)
                GW = 4                     # m chunks per grouped cast
                groups = [list(range(g, min(g + GW, nchunks)))
                          for g in range(0, nchunks, GW)]

                def emit_group(gi):
                    grp = groups[gi]
                    pm = psum_m.tile([128, 512], f32, tag="pm")
                    for jj, j in enumerate(grp):
                        co, rj = chks[j]
                        k = 0
                        for src, ws in nbr_srcs:
                            for w in ws:
                                nc.tensor.matmul(pm[:rj, jj * 128:jj * 128 + 128],
                                                 src[:, co:co + rj], w,
                                                 start=(k == 0),
                                                 stop=(k == nterm - 1))
                                k += 1
                    wd_g = len(grp) * 128
                    mhi = mp.tile([128, 512], bf16, tag="mhi")
                    if gi % 2 == 0:
                        nc.vector.tensor_copy(out=mhi[:, :wd_g], in_=pm[:, :wd_g])
                    else:
                        nc.scalar.copy(mhi[:, :wd_g], pm[:, :wd_g])
                    return mhi

                gq = [emit_group(0)]
                # self path: bf16 weights against bf16 state
                k = 0
                for src, ws in self_srcs:
                    for w in ws:
                        for a, off, wd in aggs:
                            nc.tensor.matmul(a[:, :wd], w, src[:, off:off + wd],
                                             start=(k == 0), stop=False)
                        k += 1
                for gi, grp in enumerate(groups):
                    mhi = gq.pop(0)
                    if gi + 1 < len(groups):
                        gq.append(emit_group(gi + 1))
                    for jj, j in enumerate(grp):
                        rj = chks[j][1]
                        base = moff + j * p_out
                        for a, off, wd in aggs:
                            nc.tensor.matmul(a[:, :wd],
                                             mhi[:rj, jj * 128:jj * 128 + 128],
                                             blob[:rj, base + off:base + off + wd],
                                             start=False,
                                             stop=(j == nchunks - 1))
                for a, off, wd in aggs:
                    nc.scalar.activation(out_tile[:, off:off + wd],
                                         a[:, :wd], Act.Relu,
                                         bias=bias_ap(bias_col))

            def slot_stages(s):
                """Emission closures for one slot: [embed, base1, adapt1,
                base2, ...].  Two slots are interleaved stage-by-stage so
                each layer-boundary ACT wait is hidden under the other
                slot's matmuls."""
                Pm1, P0, P1, P2, P3 = slot_sizes[s]
                lay = lays[s]
                blob = blob_t[s]
                psz = [P0, P1, P2, P3, P4]
                xT = state.tile([128, Pm1], bf16, tag=f"x{s}", name=f"x{s}")
                lat = [xT] + [state.tile([128, psz[k]], bf16, tag=f"lat{k+1}_{s}",
                                         name=f"lat{k+1}_{s}")
                              for k in range(L)]
                currs = [xT] + [state.tile([128, psz[k + 1]], bf16,
                                           tag=f"curr{k+1}_{s}",
                                           name=f"curr{k+1}_{s}")
                                for k in range(L)]
                pins = [Pm1, P0, P1, P2]
                stages = []

                def embed_stage():
                    eoff = eoffs[s]
                    ept = ep_holder[0]
                    for i_sp, (a, off, wd) in enumerate(get_aggs(Pm1)):
                        nc.tensor.matmul(a[:, :wd], embw_hi[:40, :],
                                         ept[:, eoff + off:eoff + off + wd],
                                         start=True, stop=False)
                        nc.tensor.matmul(a[:, :wd], embw_lo[:40, :],
                                         ept[:, eoff + off:eoff + off + wd],
                                         start=False, stop=True)
                        if i_sp % 2 == 0:
                            nc.vector.tensor_copy(out=xT[:, off:off + wd],
                                                  in_=a[:, :wd])
                        else:
                            nc.scalar.copy(xT[:, off:off + wd], a[:, :wd])
                stages.append(embed_stage)

                def base_stage(i):
                    def run():
                        gconv(blob, lay[f"m{i+1}"][0],
                              nbr_srcs=[(lat[i], [bwn_t[i]])],
                              self_srcs=[(lat[i], [bws_t[i]])],
                              p_in=pins[i], p_out=psz[i],
                              bias_col=BCOL_BASE + i, out_tile=lat[i + 1])
                    return run

                def adapt_stage(i):
                    def run():
                        gconv(blob, lay[f"m{i+2}"][0],
                              nbr_srcs=[(lat[i + 1], [awn_t[i][0]]),
                                        (currs[i], [awn_t[i][1]])],
                              self_srcs=[(lat[i + 1], [aws_t[i][0]]),
                                         (currs[i], [aws_t[i][1]])],
                              p_in=psz[i], p_out=psz[i + 1],
                              bias_col=BCOL_ADAPT + i, out_tile=currs[i + 1])
                        if i == L - 1:
                            nc.scalar.copy(gbT[:, s:s + 1], lat[L][:, 0:1])
                            nc.scalar.copy(gaT[:, s:s + 1], currs[L][:, 0:1])
                    return run

                # base_{i+1} ahead of adapt_i: they are independent, so in
                # the drained tail the base chain advances while the adapter
                # fills its ACT waits (critical depth ~6 instead of 8)
                stages.append(base_stage(0))
                for i in range(L - 1):
                    stages.append(base_stage(i + 1))
                    stages.append(adapt_stage(i))
                stages.append(adapt_stage(L - 1))
                return stages


            # ---- regression head (all slots at once) ----
            def whole_pass():
                load_blobs()
                streams = [slot_stages(ss) for ss in SKEW_ORDER]
                nst = len(streams[0])
                for r in range(nst + len(streams) - 1):
                    for i, stream in enumerate(streams):
                        k = r - i
                        if 0 <= k < nst:
                            stream[k]()
                emit_head()

            def head_mm(lhsT, rhs, bias_col, func):
                pm = psum_m.tile([128, 128], f32, tag="pm")
                nc.tensor.matmul(pm[:, :NG], lhsT, rhs, start=True, stop=True)
                out = state.tile([128, NG], f32, tag="hy")
                nc.scalar.activation(out[:], pm[:, :NG], func,
                                     bias=bias_ap(bias_col))
                return out

            def emit_head():
                # head with relu-free pairs constant-folded on host:
                # y = ((relu(g@Wa+ba))@Wb+bb -> relu) @ hW5 + hb5
                pm = psum_m.tile([128, 128], f32, tag="pm")
                nc.tensor.matmul(pm[:, :NG], hwa0, gbT[:],
                                 start=True, stop=False)
                nc.tensor.matmul(pm[:, :NG], hwa1, gaT[:],
                                 start=False, stop=True)
                y1 = state.tile([128, NG], f32, tag="hy")
                nc.scalar.activation(y1[:], pm[:, :NG], Act.Relu,
                                     bias=bias_ap(BCOL_HB1))
                y2 = head_mm(hwb, y1[:], BCOL_HMID + 0, Act.Relu)
                pm5 = psum_m.tile([128, 128], f32, tag="pm")
                nc.tensor.matmul(pm5[:1, :NG], hw5, y2[:],
                                 start=True, stop=True)
                yout = state.tile([1, NG], f32, tag="yout")
                nc.scalar.activation(yout[:], pm5[:1, :NG], Act.Identity,
                                     bias=bias_ap(BCOL_HB5)[:1])
                nc.sync.dma_start(y_d[:], yout[:])

            for _rep in range(reps):
                whole_pass()

    nc.compile()
    return nc


_NC_CACHE = {}
_LAST = {}


def _get_program(reps=1):
    key = (_LAST["slot_sizes"], reps)
    if key not in _NC_CACHE:
        _NC_CACHE[key] = _build_program(_LAST["slot_sizes"], reps=reps)
    return _NC_CACHE[key]


def _cones(edge, last_idx):
    """Nested cone ordering per graph.  Returns (order, sizes[n4..nm1])."""
    out = []
    for g in range(B):
        src, dst = edge[g, 0], edge[g, 1]
        order = [int(last_idx[g])]
        inset = np.zeros(N, bool)
        inset[order[0]] = True
        sizes = [1]
        for _ in range(5):
            new = np.unique(src[inset[dst]])
            new = new[~inset[new]]
            order.extend(new.tolist())
            inset[new] = True
            sizes.append(len(order))
        out.append((np.asarray(order), sizes))
    return out


def _split_hilo(a):
    hi = a.astype(BF16)
    lo = (a - hi.astype(F32)).astype(BF16)
    return hi, lo


def _prep_inputs(inputs):
    """Host-side cone construction + sharding.  Returns list of in_maps."""
    inds = np.asarray(inputs["regular_node_inds"]).astype(np.int64)
    shapes = np.asarray(inputs["regular_node_shapes"], dtype=F32)
    edge = np.asarray(inputs["edge_index"]).astype(np.int64)
    last_idx = np.asarray(inputs["last_idx"]).astype(np.int64)

    cones = _cones(edge, last_idx)
    # sort graphs by cost; slot j <- ranks [8j, 8j+8), core c <- rank 8j+c
    cost = np.array([c[1][5] + c[1][4] for c in cones])
    ranks = np.argsort(-cost, kind="stable")
    assign = ranks.reshape(NG, N_CORES)          # [slot, core] -> graph id
    slot_sizes = []
    for s in range(NG):
        gs = assign[s]
        mx = [max(cones[g][1][k] for g in gs) for k in range(6)]
        # sizes[k] = |V_{4-k}|; exact per-level maxes (Pm1,P0,P1,P2,P3)
        slot_sizes.append(tuple(int(mx[5 - l]) for l in range(5)))
    slot_sizes = tuple(slot_sizes)
    _LAST["slot_sizes"] = slot_sizes
    _LAST["assign"] = assign
    lays = [_blob_layout(s) for s in slot_sizes]

    # embed weights, hi/lo bf16 pair (exact): rows 0..31 table, 32..35 and
    # 36..39 shape_w (paired against shapes_hi / shapes_lo blob rows)
    embed_w = np.zeros((128, H), dtype=F32)
    embed_w[:VOCAB] = np.asarray(inputs["embed_table"], dtype=F32)
    embed_w[VOCAB:VOCAB + 4] = np.asarray(inputs["shape_w"], dtype=F32)
    embed_w[VOCAB + 4:VOCAB + 8] = np.asarray(inputs["shape_w"], dtype=F32)
    ehi, elo = _split_hilo(embed_w)
    # the shape_w rows must stay IDENTICAL in both copies within each of
    # hi/lo (they are), pairing: x = oh@(thi+tlo) + (shi+slo)@(swhi+swlo)
    embed_w2 = np.stack([ehi, elo], axis=1)     # [128, 2, H]

    bws2 = np.asarray(inputs["base_Wself"], dtype=F32).astype(BF16)
    bwn2 = np.asarray(inputs["base_Wnbr"], dtype=F32).astype(BF16)
    aws = np.asarray(inputs["adapt_Wself"], dtype=F32).reshape(L, 2, H, H)
    awn = np.asarray(inputs["adapt_Wnbr"], dtype=F32).reshape(L, 2, H, H)
    aws2 = np.ascontiguousarray(aws.transpose(0, 2, 1, 3)).astype(BF16)
    awn2 = np.ascontiguousarray(awn.transpose(0, 2, 1, 3)).astype(BF16)
    hW1 = np.asarray(inputs["hW1"], np.float64)
    hb1 = np.asarray(inputs["hb1"], np.float64)
    hWm = np.asarray(inputs["hWmid"], np.float64)
    hbm = np.asarray(inputs["hbmid"], np.float64)
    Wa = hW1 @ hWm[0]                       # [2H, H]
    ba = hb1 @ hWm[0] + hbm[0]
    Wb = hWm[1] @ hWm[2]                    # [H, H]
    bb = hbm[1] @ hWm[2] + hbm[2]
    hw1 = np.ascontiguousarray(
        Wa.astype(F32).reshape(2, H, H).transpose(1, 0, 2))

    biases = np.zeros((H, NBCOL), dtype=F32)
    biases[:, BCOL_BASE:BCOL_BASE + L] = np.asarray(inputs["base_b"], dtype=F32).T
    biases[:, BCOL_ADAPT:BCOL_ADAPT + L] = np.asarray(inputs["adapt_b"], dtype=F32).T
    biases[:, BCOL_HB1] = ba.astype(F32)
    biases[:, BCOL_HMID] = bb.astype(F32)
    biases[0, BCOL_HB5] = np.asarray(inputs["hb5"], dtype=F32)[0]

    WB = 2 * H + L * 6 * H
    WF = 3 * H + 1 + NBCOL
    wpack_bf = np.zeros((128, WB), dtype=BF16)
    wpack_bf[:, 0:H] = embed_w2[:, 0, :]
    wpack_bf[:, H:2 * H] = embed_w2[:, 1, :]
    for i in range(L):
        o = 2 * H + i * 6 * H
        wpack_bf[:, o:o + H] = bwn2[i]
        wpack_bf[:, o + H:o + 2 * H] = bws2[i]
        wpack_bf[:, o + 2 * H:o + 3 * H] = awn2[i][:, 0, :]
        wpack_bf[:, o + 3 * H:o + 4 * H] = awn2[i][:, 1, :]
        wpack_bf[:, o + 4 * H:o + 5 * H] = aws2[i][:, 0, :]
        wpack_bf[:, o + 5 * H:o + 6 * H] = aws2[i][:, 1, :]
    wpack_f32 = np.zeros((128, WF), dtype=F32)
    wpack_f32[:, 0:H] = Wa.astype(F32)[:H, :]
    wpack_f32[:, H:2 * H] = Wa.astype(F32)[H:, :]
    wpack_f32[:, 2 * H:3 * H] = Wb.astype(F32)
    wpack_f32[:, 3 * H:3 * H + 1] = np.asarray(inputs["hW5"], dtype=F32)
    wpack_f32[:, 3 * H + 1:] = biases
    shared = {"wpack_bf": wpack_bf, "wpack_f32": wpack_f32}
    in_maps = [dict(shared) for _ in range(N_CORES)]
    EP = sum(sz[0] for sz in slot_sizes)
    epack = [np.zeros((40, EP), dtype=BF16) for _ in range(N_CORES)]
    for s in range(NG):
        Pm1, P0, P1, P2, P3 = slot_sizes[s]
        lay = lays[s]
        for c in range(N_CORES):
            g = assign[s, c]
            order, sizes = cones[g]
            n = len(order)
            pos = np.full(N, -1, np.int64)
            pos[order] = np.arange(n)
            src, dst = edge[g, 0], edge[g, 1]
            ps, pd = pos[src], pos[dst]
            blob = np.zeros((128, lay["_total"]), dtype=FP8)
            # erhs: one-hot rows 0..31, shapes hi rows 32..35, lo rows 36..39
            eoff = sum(sz[0] for sz in slot_sizes[:s])
            erhs = np.zeros((40, Pm1), dtype=F32)
            erhs[inds[g][order], np.arange(n)] = 1.0
            shi, slo = _split_hilo(shapes[g][order].T)
            epack[c][:, eoff:eoff + Pm1] = erhs.astype(BF16)
            epack[c][VOCAB:VOCAB + 4, eoff:eoff + n] = shi[:, :n]
            epack[c][VOCAB + 4:VOCAB + 8, eoff:eoff + n] = slo[:, :n]
            for l, (pin, pout) in enumerate(lay["_dims"]):
                ncols = sizes[4 - l]   # |V_{l-1}|
                rceil = ((pin + 127) // 128) * 128
                M = np.zeros((rceil, pout), dtype=F32)
                mask = (pd >= 0) & (pd < ncols)
                np.add.at(M, (ps[mask], pd[mask]), 1.0)
                moff = lay[f"m{l + 1}"][0]
                # chunk-major on the free axis, stride pout, exact widths
                assert M.max() <= 16, "edge multiplicity exceeds fp8-exact range"
                Mt = M.astype(FP8).reshape(rceil // 128, 128, pout)
                blob[:, moff:moff + (rceil // 128) * pout] = (
                    Mt.transpose(1, 0, 2).reshape(128, -1))
            in_maps[c][f"blob{s}"] = blob
    for c in range(N_CORES):
        in_maps[c]["epack"] = epack[c]
    return in_maps


def kernel(**inputs) -> np.ndarray:
    in_maps = _prep_inputs(inputs)
    nc = _get_program()
    assign = _LAST["assign"]
    # first dispatch after a fresh compile has produced garbage before
    # (axon staging race); run twice and keep the steady-state result
    run_bass_kernel_spmd(nc, in_maps, core_ids=list(range(N_CORES)))
    res = run_bass_kernel_spmd(nc, in_maps, core_ids=list(range(N_CORES)))
    out = np.zeros((B, 1), dtype=F32)
    for c in range(N_CORES):
        yc = np.asarray(res.results[c]["y"]).reshape(NG)
        for s in range(NG):
            out[assign[s, c], 0] = yc[s]
    return out


# revision 37
# speedup vs baseline: 1.4822x; 1.4359x over previous
"""Trainium2 Bass kernel for nn_CGRegressorAdapter (GNN message passing).

Strategy (cone-restricted):
  - The regression head only reads ONE node per graph (last_idx), so each
    layer of the 8-layer GNN stack only needs the node's influence cone:
    V_4={v} at the top, growing by in-neighborhoods down to V_{-1} (~1400
    nodes max) at the embed layer.  Host prep computes nested cone
    orderings (V_{k+1} is a prefix of V_k) and compacted adjacency slices
    M_l = A[V_{l-2}, V_{l-1}] (edge counts, exact in bf16).
  - Data-parallel over B=32 graphs: 8 cores x 4 slots.  Graphs are sorted
    by cone cost; slot j holds ranks [8j, 8j+8) and is sized to that
    quartile's EXACT per-level maxes (no 128-padding on free axes; the
    contraction runs 128-row chunks with a partial last chunk), so the
    small top layers cost almost nothing.
  - Adjacency slices ship as per-slot fp8-e4m3 blobs (edge counts <=16
    are exact) upcast to bf16 in-flight by SWDGE casting DMAs; embed
    inputs for all slots ship as one [40, sum Pm1] bf16 pack (embW rows
    >=40 are zero, so the matmul contracts 40 partitions); weights ship
    as two packed tiles.  DMA priority: embed weights, embed inputs, GNN
    weights, head weights, M blobs smallest slot first (big slots split
    so their L1 can start on the first half).
  - Per slot: embed (bf16 hi/lo one-hot matmul, f32-exact), 4 base + 4
    adapter GraphConvs, all matmuls single-bf16 (states bf16, weights
    bf16), f32 PSUM accumulate.  m-chunks are batched 4-at-a-time in one
    [128,512] PSUM tile and cast with a single DVE/ACT copy (alternating
    engines).  Measured end-to-end rel err 8.4e-3 vs the 2e-2 gate.
  - The four slot streams are emitted in a skewed staircase (stream i
    runs i stages behind) so layer-boundary ACT waits hide under other
    slots' matmuls and PSUM agg buffers are never oversubscribed; within
    a stream, base layer i+1 is emitted before adapter layer i (they are
    independent) to shorten the drained-tail critical chain.
  - Nested prefix ordering makes the self path a plain prefix slice and
    the final extraction column 0.  Regression head (relu-free layer
    pairs constant-folded on host) on-chip in f32.
"""
import numpy as np
import ml_dtypes

import concourse.bass as bass
import concourse.mybir as mybir
from concourse import bacc
from concourse.bass import ts
from concourse.bass_utils import run_bass_kernel_spmd
from concourse.tile import TileContext

BF16 = ml_dtypes.bfloat16
FP8 = ml_dtypes.float8_e4m3
F32 = np.float32

B, N, E, H, L, VOCAB = 32, 2048, 8192, 128, 4, 32
N_CORES = 8
NG = B // N_CORES          # graphs (slots) per core
dt = mybir.dt
Alu = mybir.AluOpType
Act = mybir.ActivationFunctionType

# bias column indices in the packed bias tile
BCOL_BASE = 0      # 0..3  base_b
BCOL_ADAPT = 4     # 4..7  adapt_b
BCOL_HB1 = 8
BCOL_HMID = 9      # 9..11
BCOL_HB5 = 12
NBCOL = 16


def _ceil128(x):
    return max(128, (int(x) + 127) // 128 * 128)


def _chunks(n):
    """[(col_off, rows)] covering n in 128-row chunks, last may be partial."""
    return [(j * 128, min(128, n - j * 128)) for j in range((n + 127) // 128)]


def _spans(width, maxw=512):
    out = []
    off = 0
    while off < width:
        w = min(maxw, width - off)
        out.append((off, w))
        off += w
    return out


def _blob_layout(sizes):
    """Free-axis offsets of the per-slot bf16 blob [128, W].
    Sections: erhs [128, Pm1], then M_l as [128, (pin/128)*pout] l=1..5."""
    Pm1, P0, P1, P2, P3 = sizes
    P4 = 1
    dims = [(Pm1, P0), (P0, P1), (P1, P2), (P2, P3), (P3, P4)]
    lay = {}
    off = 0
    for l, (pin, pout) in enumerate(dims):
        w = len(_chunks(pin)) * pout
        lay[f"m{l + 1}"] = (off, w)
        off += w
    lay["_total"] = off
    lay["_dims"] = dims
    return lay


DMA_ORDER = (3, 2, 1, 0)
MP_BUFS = 6
PSUM_AGG_BUFS = 2
PSUM_M_BUFS = 4
SKEW_ORDER = (3, 2, 1, 0)


def _build_program(slot_sizes, reps=1):
    """slot_sizes: tuple of 4 tuples (Pm1, P0, P1, P2, P3) padded sizes.
    reps>1 repeats the whole body serially (timing: slope removes
    dispatch overhead)."""
    nc = bacc.Bacc("TRN2", target_bir_lowering=False, debug=False,
                   num_devices=N_CORES)
    f32, bf16 = dt.float32, dt.bfloat16
    P4 = 1
    lays = [_blob_layout(s) for s in slot_sizes]

    # all weights packed into two tiles: bf16 (embed + GNN) and f32 (head)
    WB = 2 * H + L * 6 * H          # embw hi/lo + per layer bwn,bws,awn2,aws2
    WF = 3 * H + 1 + NBCOL          # hwa(2H) + hwb(H) + hw5(1) + biases
    wb_d = nc.declare_dram_parameter("wpack_bf", [128, WB], bf16, isOutput=False)
    wf_d = nc.declare_dram_parameter("wpack_f32", [128, WF], f32, isOutput=False)
    EP = sum(sz[0] for sz in slot_sizes)      # all slots' erhs, 40 rows
    ep_d = nc.declare_dram_parameter("epack", [40, EP], bf16, isOutput=False)
    eoffs = [sum(sz[0] for sz in slot_sizes[:s]) for s in range(NG)]
    fp8 = dt.float8e4
    blob_d = [nc.declare_dram_parameter(f"blob{s}", [128, lays[s]["_total"]],
                                        fp8, isOutput=False)
              for s in range(NG)]
    y_d = nc.declare_dram_parameter("y", [1, NG], f32, isOutput=True)

    with TileContext(nc) as tc:
        with (
            tc.tile_pool(name="const", bufs=1) as const,
            tc.tile_pool(name="state", bufs=1) as state,
            tc.tile_pool(name="mp", bufs=MP_BUFS) as mp,
            tc.tile_pool(name="psum_agg", bufs=PSUM_AGG_BUFS, space="PSUM") as psum_agg,
            tc.tile_pool(name="psum_m", bufs=PSUM_M_BUFS, space="PSUM") as psum_m,
        ):
            # ---- all input DMAs issued up front (prefetch) ----
            blob_t = [None] * NG
            ep_holder = [None]

            wb_t = const.tile([128, WB], bf16)
            wf_t = const.tile([128, WF], f32)
            consts_loaded = [False]

            def load_blobs():
                # DMA priority: embed weights, embed inputs, GNN weights,
                # head weights, then M blobs smallest slot first
                if not consts_loaded[0]:
                    nc.sync.dma_start(wb_t[:, :2 * H], wb_d[:, :2 * H])
                ep_holder[0] = state.tile([40, EP], bf16, tag="epack",
                                          name="epack")
                nc.sync.dma_start(ep_holder[0][:], ep_d[:])
                if not consts_loaded[0]:
                    nc.sync.dma_start(wb_t[:, 2 * H:], wb_d[:, 2 * H:])
                    nc.sync.dma_start(wf_t[:], wf_d[:])
                    consts_loaded[0] = True
                for s in DMA_ORDER:
                    blob_t[s] = state.tile([128, lays[s]["_total"]], bf16,
                                           tag=f"blob{s}", name=f"blob{s}")
                    half = (lays[s]["m1"][1] // 2 // 128) * 128
                    # SWDGE casting DMA: fp8 in HBM (counts are exact),
                    # bf16 in SBUF — halves the dominant DMA traffic
                    if half == 0:
                        nc.gpsimd.dma_start(blob_t[s][:], blob_d[s][:])
                    else:
                        # split so the slot's L1 can start on the first half
                        nc.gpsimd.dma_start(blob_t[s][:, :half],
                                            blob_d[s][:, :half])
                        nc.gpsimd.dma_start(blob_t[s][:, half:],
                                            blob_d[s][:, half:])
            embw_hi = wb_t[:, 0:H]
            embw_lo = wb_t[:, H:2 * H]
            bwn_t, bws_t, awn_t, aws_t = [], [], [], []
            for i in range(L):
                o = 2 * H + i * 6 * H
                bwn_t.append(wb_t[:, o:o + H])
                bws_t.append(wb_t[:, o + H:o + 2 * H])
                awn_t.append((wb_t[:, o + 2 * H:o + 3 * H],
                              wb_t[:, o + 3 * H:o + 4 * H]))
                aws_t.append((wb_t[:, o + 4 * H:o + 5 * H],
                              wb_t[:, o + 5 * H:o + 6 * H]))
            hwa0 = wf_t[:, 0:H]
            hwa1 = wf_t[:, H:2 * H]
            hwb = wf_t[:, 2 * H:3 * H]
            hw5 = wf_t[:, 3 * H:3 * H + 1]
            BOFF = 3 * H + 1

            def bias_ap(col):
                return wf_t[:, BOFF + col:BOFF + col + 1]

            gbT = state.tile([128, NG], f32, tag="gb")
            gaT = state.tile([128, NG], f32, tag="ga")

            # per-span PSUM agg tiles are fixed [128,512] and reused by tag
            def get_aggs(width):
                return [(psum_agg.tile([128, 512], f32, tag=f"agg{i % 2}",
                                       name=f"agg{i % 2}"), off, w)
                        for i, (off, w) in enumerate(_spans(width))]

            def gconv(blob, moff, nbr_srcs, self_srcs, p_in, p_out, bias_col,
                      out_tile):
                """nbr_srcs: list of (stateT [128,p_in] bf16, [W_hi, W_lo]
                rhs aps).  self_srcs: list of (stateT, [Wself hi/lo lhsT
                aps]).  blob[:, moff+j*p_out :] holds the bf16 count slice
                for chunk j."""
                chks = _chunks(p_in)       # [(col_off, rows)], exact sizes
                nchunks = len(chks)
                aggs = get_aggs(p_out)
                nterm = sum(len(ws) for _, ws in nbr_srcs)
                GW = 4                     # m chunks per grouped cast
                groups = [list(range(g, min(g + GW, nchunks)))
                          for g in range(0, nchunks, GW)]

                def emit_group(gi):
                    grp = groups[gi]
                    pm = psum_m.tile([128, 512], f32, tag="pm")
                    for jj, j in enumerate(grp):
                        co, rj = chks[j]
                        k = 0
                        for src, ws in nbr_srcs:
                            for w in ws:
                                nc.tensor.matmul(pm[:rj, jj * 128:jj * 128 + 128],
                                                 src[:, co:co + rj], w,
                                                 start=(k == 0),
                                                 stop=(k == nterm - 1))
                                k += 1
                    wd_g = len(grp) * 128
                    mhi = mp.tile([128, 512], bf16, tag="mhi")
                    if gi % 2 == 0:
                        nc.vector.tensor_copy(out=mhi[:, :wd_g], in_=pm[:, :wd_g])
                    else:
                        nc.scalar.copy(mhi[:, :wd_g], pm[:, :wd_g])
                    return mhi

                gq = [emit_group(0)]
                # self path: bf16 weights against bf16 state
                k = 0
                for src, ws in self_srcs:
                    for w in ws:
                        for a, off, wd in aggs:
                            nc.tensor.matmul(a[:, :wd], w, src[:, off:off + wd],
                                             start=(k == 0), stop=False)
                        k += 1
                for gi, grp in enumerate(groups):
                    mhi = gq.pop(0)
                    if gi + 1 < len(groups):
                        gq.append(emit_group(gi + 1))
                    for jj, j in enumerate(grp):
                        rj = chks[j][1]
                        base = moff + j * p_out
                        for a, off, wd in aggs:
                            nc.tensor.matmul(a[:, :wd],
                                             mhi[:rj, jj * 128:jj * 128 + 128],
                                             blob[:rj, base + off:base + off + wd],
                                             start=False,
                                             stop=(j == nchunks - 1))
                for a, off, wd in aggs:
                    nc.scalar.activation(out_tile[:, off:off + wd],
                                         a[:, :wd], Act.Relu,
                                         bias=bias_ap(bias_col))

            def slot_stages(s):
                """Emission closures for one slot: [embed, base1, adapt1,
                base2, ...].  Two slots are interleaved stage-by-stage so
                each layer-boundary ACT wait is hidden under the other
                slot's matmuls."""
                Pm1, P0, P1, P2, P3 = slot_sizes[s]
                lay = lays[s]
                blob = blob_t[s]
                psz = [P0, P1, P2, P3, P4]
                xT = state.tile([128, Pm1], bf16, tag=f"x{s}", name=f"x{s}")
                lat = [xT] + [state.tile([128, psz[k]], bf16, tag=f"lat{k+1}_{s}",
                                         name=f"lat{k+1}_{s}")
                              for k in range(L)]
                currs = [xT] + [state.tile([128, psz[k + 1]], bf16,
                                           tag=f"curr{k+1}_{s}",
                                           name=f"curr{k+1}_{s}")
                                for k in range(L)]
                pins = [Pm1, P0, P1, P2]
                stages = []

                def embed_stage():
                    eoff = eoffs[s]
                    ept = ep_holder[0]
                    for i_sp, (a, off, wd) in enumerate(get_aggs(Pm1)):
                        nc.tensor.matmul(a[:, :wd], embw_hi[:40, :],
                                         ept[:, eoff + off:eoff + off + wd],
                                         start=True, stop=False)
                        nc.tensor.matmul(a[:, :wd], embw_lo[:40, :],
                                         ept[:, eoff + off:eoff + off + wd],
                                         start=False, stop=True)
                        if i_sp % 2 == 0:
                            nc.vector.tensor_copy(out=xT[:, off:off + wd],
                                                  in_=a[:, :wd])
                        else:
                            nc.scalar.copy(xT[:, off:off + wd], a[:, :wd])
                stages.append(embed_stage)

                def base_stage(i):
                    def run():
                        gconv(blob, lay[f"m{i+1}"][0],
                              nbr_srcs=[(lat[i], [bwn_t[i]])],
                              self_srcs=[(lat[i], [bws_t[i]])],
                              p_in=pins[i], p_out=psz[i],
                              bias_col=BCOL_BASE + i, out_tile=lat[i + 1])
                    return run

                def adapt_stage(i):
                    def run():
                        # the last adapter output is only read at column 0
                        # (the head input): write it straight into gaT and
                        # skip the extraction hop on the critical tail
                        out_t = currs[i + 1] if i < L - 1 else gaT[:, s:s + 1]
                        gconv(blob, lay[f"m{i+2}"][0],
                              nbr_srcs=[(lat[i + 1], [awn_t[i][0]]),
                                        (currs[i], [awn_t[i][1]])],
                              self_srcs=[(lat[i + 1], [aws_t[i][0]]),
                                         (currs[i], [aws_t[i][1]])],
                              p_in=psz[i], p_out=psz[i + 1],
                              bias_col=BCOL_ADAPT + i, out_tile=out_t)
                        if i == L - 1:
                            nc.scalar.copy(gbT[:, s:s + 1], lat[L][:, 0:1])
                    return run

                # base_{i+1} ahead of adapt_i: they are independent, so in
                # the drained tail the base chain advances while the adapter
                # fills its ACT waits (critical depth ~6 instead of 8)
                stages.append(base_stage(0))
                for i in range(L - 1):
                    stages.append(base_stage(i + 1))
                    stages.append(adapt_stage(i))
                stages.append(adapt_stage(L - 1))
                return stages


            # ---- regression head (all slots at once) ----
            def whole_pass():
                load_blobs()
                streams = [slot_stages(ss) for ss in SKEW_ORDER]
                nst = len(streams[0])
                for r in range(nst + len(streams) - 1):
                    for i, stream in enumerate(streams):
                        k = r - i
                        if 0 <= k < nst:
                            stream[k]()
                emit_head()

            def head_mm(lhsT, rhs, bias_col, func):
                pm = psum_m.tile([128, 128], f32, tag="pm")
                nc.tensor.matmul(pm[:, :NG], lhsT, rhs, start=True, stop=True)
                out = state.tile([128, NG], f32, tag="hy")
                nc.scalar.activation(out[:], pm[:, :NG], func,
                                     bias=bias_ap(bias_col))
                return out

            def emit_head():
                # head with relu-free pairs constant-folded on host:
                # y = ((relu(g@Wa+ba))@Wb+bb -> relu) @ hW5 + hb5
                pm = psum_m.tile([128, 128], f32, tag="pm")
                nc.tensor.matmul(pm[:, :NG], hwa0, gbT[:],
                                 start=True, stop=False)
                nc.tensor.matmul(pm[:, :NG], hwa1, gaT[:],
                                 start=False, stop=True)
                y1 = state.tile([128, NG], f32, tag="hy")
                nc.scalar.activation(y1[:], pm[:, :NG], Act.Relu,
                                     bias=bias_ap(BCOL_HB1))
                y2 = head_mm(hwb, y1[:], BCOL_HMID + 0, Act.Relu)
                pm5 = psum_m.tile([128, 128], f32, tag="pm")
                nc.tensor.matmul(pm5[:1, :NG], hw5, y2[:],
                                 start=True, stop=True)
                yout = state.tile([1, NG], f32, tag="yout")
                nc.scalar.activation(yout[:], pm5[:1, :NG], Act.Identity,
                                     bias=bias_ap(BCOL_HB5)[:1])
                nc.sync.dma_start(y_d[:], yout[:])

            for _rep in range(reps):
                whole_pass()

    nc.compile()
    return nc


_NC_CACHE = {}
_LAST = {}


def _get_program(reps=1):
    key = (_LAST["slot_sizes"], reps)
    if key not in _NC_CACHE:
        _NC_CACHE[key] = _build_program(_LAST["slot_sizes"], reps=reps)
    return _NC_CACHE[key]


def _cones(edge, last_idx):
    """Nested cone ordering per graph.  Returns (order, sizes[n4..nm1])."""
    out = []
    for g in range(B):
        src, dst = edge[g, 0], edge[g, 1]
        order = [int(last_idx[g])]
        inset = np.zeros(N, bool)
        inset[order[0]] = True
        sizes = [1]
        for _ in range(5):
            new = np.unique(src[inset[dst]])
            new = new[~inset[new]]
            order.extend(new.tolist())
            inset[new] = True
            sizes.append(len(order))
        out.append((np.asarray(order), sizes))
    return out


def _split_hilo(a):
    hi = a.astype(BF16)
    lo = (a - hi.astype(F32)).astype(BF16)
    return hi, lo


def _prep_inputs(inputs):
    """Host-side cone construction + sharding.  Returns list of in_maps."""
    inds = np.asarray(inputs["regular_node_inds"]).astype(np.int64)
    shapes = np.asarray(inputs["regular_node_shapes"], dtype=F32)
    edge = np.asarray(inputs["edge_index"]).astype(np.int64)
    last_idx = np.asarray(inputs["last_idx"]).astype(np.int64)

    cones = _cones(edge, last_idx)
    # sort graphs by cost; slot j <- ranks [8j, 8j+8), core c <- rank 8j+c
    cost = np.array([c[1][5] + c[1][4] for c in cones])
    ranks = np.argsort(-cost, kind="stable")
    assign = ranks.reshape(NG, N_CORES)          # [slot, core] -> graph id
    slot_sizes = []
    for s in range(NG):
        gs = assign[s]
        mx = [max(cones[g][1][k] for g in gs) for k in range(6)]
        # sizes[k] = |V_{4-k}|; exact per-level maxes (Pm1,P0,P1,P2,P3)
        slot_sizes.append(tuple(int(mx[5 - l]) for l in range(5)))
    slot_sizes = tuple(slot_sizes)
    _LAST["slot_sizes"] = slot_sizes
    _LAST["assign"] = assign
    lays = [_blob_layout(s) for s in slot_sizes]

    # embed weights, hi/lo bf16 pair (exact): rows 0..31 table, 32..35 and
    # 36..39 shape_w (paired against shapes_hi / shapes_lo blob rows)
    embed_w = np.zeros((128, H), dtype=F32)
    embed_w[:VOCAB] = np.asarray(inputs["embed_table"], dtype=F32)
    embed_w[VOCAB:VOCAB + 4] = np.asarray(inputs["shape_w"], dtype=F32)
    embed_w[VOCAB + 4:VOCAB + 8] = np.asarray(inputs["shape_w"], dtype=F32)
    ehi, elo = _split_hilo(embed_w)
    # the shape_w rows must stay IDENTICAL in both copies within each of
    # hi/lo (they are), pairing: x = oh@(thi+tlo) + (shi+slo)@(swhi+swlo)
    embed_w2 = np.stack([ehi, elo], axis=1)     # [128, 2, H]

    bws2 = np.asarray(inputs["base_Wself"], dtype=F32).astype(BF16)
    bwn2 = np.asarray(inputs["base_Wnbr"], dtype=F32).astype(BF16)
    aws = np.asarray(inputs["adapt_Wself"], dtype=F32).reshape(L, 2, H, H)
    awn = np.asarray(inputs["adapt_Wnbr"], dtype=F32).reshape(L, 2, H, H)
    aws2 = np.ascontiguousarray(aws.transpose(0, 2, 1, 3)).astype(BF16)
    awn2 = np.ascontiguousarray(awn.transpose(0, 2, 1, 3)).astype(BF16)
    hW1 = np.asarray(inputs["hW1"], np.float64)
    hb1 = np.asarray(inputs["hb1"], np.float64)
    hWm = np.asarray(inputs["hWmid"], np.float64)
    hbm = np.asarray(inputs["hbmid"], np.float64)
    Wa = hW1 @ hWm[0]                       # [2H, H]
    ba = hb1 @ hWm[0] + hbm[0]
    Wb = hWm[1] @ hWm[2]                    # [H, H]
    bb = hbm[1] @ hWm[2] + hbm[2]
    hw1 = np.ascontiguousarray(
        Wa.astype(F32).reshape(2, H, H).transpose(1, 0, 2))

    biases = np.zeros((H, NBCOL), dtype=F32)
    biases[:, BCOL_BASE:BCOL_BASE + L] = np.asarray(inputs["base_b"], dtype=F32).T
    biases[:, BCOL_ADAPT:BCOL_ADAPT + L] = np.asarray(inputs["adapt_b"], dtype=F32).T
    biases[:, BCOL_HB1] = ba.astype(F32)
    biases[:, BCOL_HMID] = bb.astype(F32)
    biases[0, BCOL_HB5] = np.asarray(inputs["hb5"], dtype=F32)[0]

    WB = 2 * H + L * 6 * H
    WF = 3 * H + 1 + NBCOL
    wpack_bf = np.zeros((128, WB), dtype=BF16)
    wpack_bf[:, 0:H] = embed_w2[:, 0, :]
    wpack_bf[:, H:2 * H] = embed_w2[:, 1, :]
    for i in range(L):
        o = 2 * H + i * 6 * H
        wpack_bf[:, o:o + H] = bwn2[i]
        wpack_bf[:, o + H:o + 2 * H] = bws2[i]
        wpack_bf[:, o + 2 * H:o + 3 * H] = awn2[i][:, 0, :]
        wpack_bf[:, o + 3 * H:o + 4 * H] = awn2[i][:, 1, :]
        wpack_bf[:, o + 4 * H:o + 5 * H] = aws2[i][:, 0, :]
        wpack_bf[:, o + 5 * H:o + 6 * H] = aws2[i][:, 1, :]
    wpack_f32 = np.zeros((128, WF), dtype=F32)
    wpack_f32[:, 0:H] = Wa.astype(F32)[:H, :]
    wpack_f32[:, H:2 * H] = Wa.astype(F32)[H:, :]
    wpack_f32[:, 2 * H:3 * H] = Wb.astype(F32)
    wpack_f32[:, 3 * H:3 * H + 1] = np.asarray(inputs["hW5"], dtype=F32)
    wpack_f32[:, 3 * H + 1:] = biases
    shared = {"wpack_bf": wpack_bf, "wpack_f32": wpack_f32}
    in_maps = [dict(shared) for _ in range(N_CORES)]
    EP = sum(sz[0] for sz in slot_sizes)
    epack = [np.zeros((40, EP), dtype=BF16) for _ in range(N_CORES)]
    for s in range(NG):
        Pm1, P0, P1, P2, P3 = slot_sizes[s]
        lay = lays[s]
        for c in range(N_CORES):
            g = assign[s, c]
            order, sizes = cones[g]
            n = len(order)
            pos = np.full(N, -1, np.int64)
            pos[order] = np.arange(n)
            src, dst = edge[g, 0], edge[g, 1]
            ps, pd = pos[src], pos[dst]
            blob = np.zeros((128, lay["_total"]), dtype=FP8)
            # erhs: one-hot rows 0..31, shapes hi rows 32..35, lo rows 36..39
            eoff = sum(sz[0] for sz in slot_sizes[:s])
            erhs = np.zeros((40, Pm1), dtype=F32)
            erhs[inds[g][order], np.arange(n)] = 1.0
            shi, slo = _split_hilo(shapes[g][order].T)
            epack[c][:, eoff:eoff + Pm1] = erhs.astype(BF16)
            epack[c][VOCAB:VOCAB + 4, eoff:eoff + n] = shi[:, :n]
            epack[c][VOCAB + 4:VOCAB + 8, eoff:eoff + n] = slo[:, :n]
            for l, (pin, pout) in enumerate(lay["_dims"]):
                ncols = sizes[4 - l]   # |V_{l-1}|
                rceil = ((pin + 127) // 128) * 128
                M = np.zeros((rceil, pout), dtype=F32)
                mask = (pd >= 0) & (pd < ncols)
                np.add.at(M, (ps[mask], pd[mask]), 1.0)
                moff = lay[f"m{l + 1}"][0]
                # chunk-major on the free axis, stride pout, exact widths
                assert M.max() <= 16, "edge multiplicity exceeds fp8-exact range"
                Mt = M.astype(FP8).reshape(rceil // 128, 128, pout)
                blob[:, moff:moff + (rceil // 128) * pout] = (
                    Mt.transpose(1, 0, 2).reshape(128, -1))
            in_maps[c][f"blob{s}"] = blob
    for c in range(N_CORES):
        in_maps[c]["epack"] = epack[c]
    return in_maps


def kernel(**inputs) -> np.ndarray:
    in_maps = _prep_inputs(inputs)
    nc = _get_program()
    assign = _LAST["assign"]
    # first dispatch after a fresh compile has produced garbage before
    # (axon staging race); run twice and keep the steady-state result
    run_bass_kernel_spmd(nc, in_maps, core_ids=list(range(N_CORES)))
    res = run_bass_kernel_spmd(nc, in_maps, core_ids=list(range(N_CORES)))
    out = np.zeros((B, 1), dtype=F32)
    for c in range(N_CORES):
        yc = np.asarray(res.results[c]["y"]).reshape(NG)
        for s in range(NG):
            out[assign[s, c], 0] = yc[s]
    return out


# revision 39
# speedup vs baseline: 1.5615x; 1.0535x over previous
"""Trainium2 Bass kernel for nn_CGRegressorAdapter (GNN message passing).

Strategy (cone-restricted):
  - The regression head only reads ONE node per graph (last_idx), so each
    layer of the 8-layer GNN stack only needs the node's influence cone:
    V_4={v} at the top, growing by in-neighborhoods down to V_{-1} (~1400
    nodes max) at the embed layer.  Host prep computes nested cone
    orderings (V_{k+1} is a prefix of V_k) and compacted adjacency slices
    M_l = A[V_{l-2}, V_{l-1}] (edge counts, exact in bf16).
  - Data-parallel over B=32 graphs: 8 cores x 4 slots.  Graphs are sorted
    by cone cost; slot j holds ranks [8j, 8j+8) and is sized to that
    quartile's EXACT per-level maxes (no 128-padding on free axes; the
    contraction runs 128-row chunks with a partial last chunk), so the
    small top layers cost almost nothing.
  - Adjacency slices ship as per-slot fp8-e4m3 blobs (edge counts <=16
    are exact) upcast to bf16 in-flight by SWDGE casting DMAs; embed
    inputs for all slots ship as one [40, sum Pm1] bf16 pack (embW rows
    >=40 are zero, so the matmul contracts 40 partitions); weights ship
    as two packed tiles.  DMA priority: embed weights, embed inputs, GNN
    weights, head weights, M blobs smallest slot first (big slots split
    so their L1 can start on the first half).
  - Per slot: embed (bf16 hi/lo one-hot matmul, f32-exact), 4 base + 4
    adapter GraphConvs, all matmuls single-bf16 (states bf16, weights
    bf16), f32 PSUM accumulate.  m-chunks are batched 4-at-a-time in one
    [128,512] PSUM tile and cast with a single DVE/ACT copy (alternating
    engines).  The last adapter layer's ACT writes its single output
    column straight into the f32 head-input tile (no extraction hop, no
    bf16 rounding).  Measured end-to-end rel err 6.1e-3 vs the 2e-2 gate.
  - The four slot streams are emitted in a skewed staircase (stream i
    runs i stages behind) so layer-boundary ACT waits hide under other
    slots' matmuls and PSUM agg buffers are never oversubscribed; within
    a stream, base layer i+1 is emitted before adapter layer i (they are
    independent) to shorten the drained-tail critical chain.
  - Nested prefix ordering makes the self path a plain prefix slice and
    the final extraction column 0.  Regression head (relu-free layer
    pairs constant-folded on host) on-chip in f32.
"""
import numpy as np
import ml_dtypes

import concourse.bass as bass
import concourse.mybir as mybir
from concourse import bacc
from concourse.bass import ts
from concourse.bass_utils import run_bass_kernel_spmd
from concourse.tile import TileContext

BF16 = ml_dtypes.bfloat16
FP8 = ml_dtypes.float8_e4m3
F32 = np.float32

B, N, E, H, L, VOCAB = 32, 2048, 8192, 128, 4, 32
N_CORES = 8
NG = B // N_CORES          # graphs (slots) per core
dt = mybir.dt
Alu = mybir.AluOpType
Act = mybir.ActivationFunctionType

# bias column indices in the packed bias tile
BCOL_BASE = 0      # 0..3  base_b
BCOL_ADAPT = 4     # 4..7  adapt_b
BCOL_HB1 = 8
BCOL_HMID = 9      # 9..11
BCOL_HB5 = 12
NBCOL = 16


def _ceil128(x):
    return max(128, (int(x) + 127) // 128 * 128)


def _chunks(n):
    """[(col_off, rows)] covering n in 128-row chunks, last may be partial."""
    return [(j * 128, min(128, n - j * 128)) for j in range((n + 127) // 128)]


def _spans(width, maxw=512):
    out = []
    off = 0
    while off < width:
        w = min(maxw, width - off)
        out.append((off, w))
        off += w
    return out


def _blob_layout(sizes):
    """Free-axis offsets of the per-slot bf16 blob [128, W].
    Sections: erhs [128, Pm1], then M_l as [128, (pin/128)*pout] l=1..5."""
    Pm1, P0, P1, P2, P3 = sizes
    P4 = 1
    dims = [(Pm1, P0), (P0, P1), (P1, P2), (P2, P3), (P3, P4)]
    lay = {}
    off = 0
    for l, (pin, pout) in enumerate(dims):
        w = len(_chunks(pin)) * pout
        lay[f"m{l + 1}"] = (off, w)
        off += w
    lay["_total"] = off
    lay["_dims"] = dims
    return lay


DMA_ORDER = (3, 2, 1, 0)
MP_BUFS = 6
PSUM_AGG_BUFS = 2
PSUM_M_BUFS = 4
SKEW_ORDER = (3, 2, 1, 0)


def _build_program(slot_sizes, reps=1):
    """slot_sizes: tuple of 4 tuples (Pm1, P0, P1, P2, P3) padded sizes.
    reps>1 repeats the whole body serially (timing: slope removes
    dispatch overhead)."""
    nc = bacc.Bacc("TRN2", target_bir_lowering=False, debug=False,
                   num_devices=N_CORES)
    f32, bf16 = dt.float32, dt.bfloat16
    P4 = 1
    lays = [_blob_layout(s) for s in slot_sizes]

    # all weights packed into two tiles: bf16 (embed + GNN) and f32 (head)
    WB = 2 * H + L * 6 * H          # embw hi/lo + per layer bwn,bws,awn2,aws2
    WF = 3 * H + 1 + NBCOL          # hwa(2H) + hwb(H) + hw5(1) + biases
    wb_d = nc.declare_dram_parameter("wpack_bf", [128, WB], bf16, isOutput=False)
    wf_d = nc.declare_dram_parameter("wpack_f32", [128, WF], f32, isOutput=False)
    EP = sum(sz[0] for sz in slot_sizes)      # all slots' erhs, 40 rows
    ep_d = nc.declare_dram_parameter("epack", [40, EP], bf16, isOutput=False)
    eoffs = [sum(sz[0] for sz in slot_sizes[:s]) for s in range(NG)]
    fp8 = dt.float8e4
    blob_d = [nc.declare_dram_parameter(f"blob{s}", [128, lays[s]["_total"]],
                                        fp8, isOutput=False)
              for s in range(NG)]
    y_d = nc.declare_dram_parameter("y", [1, NG], f32, isOutput=True)

    with TileContext(nc) as tc:
        with (
            tc.tile_pool(name="const", bufs=1) as const,
            tc.tile_pool(name="state", bufs=1) as state,
            tc.tile_pool(name="mp", bufs=MP_BUFS) as mp,
            tc.tile_pool(name="psum_agg", bufs=PSUM_AGG_BUFS, space="PSUM") as psum_agg,
            tc.tile_pool(name="psum_m", bufs=PSUM_M_BUFS, space="PSUM") as psum_m,
        ):
            # ---- all input DMAs issued up front (prefetch) ----
            blob_t = [None] * NG
            ep_holder = [None]

            wb_t = const.tile([128, WB], bf16)
            wf_t = const.tile([128, WF], f32)
            consts_loaded = [False]

            def load_blobs():
                # DMA priority: embed weights, embed inputs, GNN weights,
                # head weights, then M blobs smallest slot first
                if not consts_loaded[0]:
                    nc.sync.dma_start(wb_t[:, :2 * H], wb_d[:, :2 * H])
                ep_holder[0] = state.tile([40, EP], bf16, tag="epack",
                                          name="epack")
                nc.sync.dma_start(ep_holder[0][:], ep_d[:])
                if not consts_loaded[0]:
                    nc.sync.dma_start(wb_t[:, 2 * H:], wb_d[:, 2 * H:])
                    nc.sync.dma_start(wf_t[:], wf_d[:])
                    consts_loaded[0] = True
                for s in DMA_ORDER:
                    blob_t[s] = state.tile([128, lays[s]["_total"]], bf16,
                                           tag=f"blob{s}", name=f"blob{s}")
                    half = (lays[s]["m1"][1] // 2 // 128) * 128
                    # SWDGE casting DMA: fp8 in HBM (counts are exact),
                    # bf16 in SBUF — halves the dominant DMA traffic
                    if half == 0:
                        nc.gpsimd.dma_start(blob_t[s][:], blob_d[s][:])
                    else:
                        # split so the slot's L1 can start on the first half
                        nc.gpsimd.dma_start(blob_t[s][:, :half],
                                            blob_d[s][:, :half])
                        nc.gpsimd.dma_start(blob_t[s][:, half:],
                                            blob_d[s][:, half:])
            embw_hi = wb_t[:, 0:H]
            embw_lo = wb_t[:, H:2 * H]
            bwn_t, bws_t, awn_t, aws_t = [], [], [], []
            for i in range(L):
                o = 2 * H + i * 6 * H
                bwn_t.append(wb_t[:, o:o + H])
                bws_t.append(wb_t[:, o + H:o + 2 * H])
                awn_t.append((wb_t[:, o + 2 * H:o + 3 * H],
                              wb_t[:, o + 3 * H:o + 4 * H]))
                aws_t.append((wb_t[:, o + 4 * H:o + 5 * H],
                              wb_t[:, o + 5 * H:o + 6 * H]))
            hwa0 = wf_t[:, 0:H]
            hwa1 = wf_t[:, H:2 * H]
            hwb = wf_t[:, 2 * H:3 * H]
            hw5 = wf_t[:, 3 * H:3 * H + 1]
            BOFF = 3 * H + 1

            def bias_ap(col):
                return wf_t[:, BOFF + col:BOFF + col + 1]

            gbT = state.tile([128, NG], f32, tag="gb")
            gaT = state.tile([128, NG], f32, tag="ga")

            # per-span PSUM agg tiles are fixed [128,512] and reused by tag
            def get_aggs(width):
                return [(psum_agg.tile([128, 512], f32, tag=f"agg{i % 2}",
                                       name=f"agg{i % 2}"), off, w)
                        for i, (off, w) in enumerate(_spans(width))]

            def gconv(blob, moff, nbr_srcs, self_srcs, p_in, p_out, bias_col,
                      out_tile, col0_out=None):
                """nbr_srcs: list of (stateT [128,p_in] bf16, [W_hi, W_lo]
                rhs aps).  self_srcs: list of (stateT, [Wself hi/lo lhsT
                aps]).  blob[:, moff+j*p_out :] holds the bf16 count slice
                for chunk j."""
                chks = _chunks(p_in)       # [(col_off, rows)], exact sizes
                nchunks = len(chks)
                aggs = get_aggs(p_out)
                nterm = sum(len(ws) for _, ws in nbr_srcs)
                GW = 4                     # m chunks per grouped cast
                groups = [list(range(g, min(g + GW, nchunks)))
                          for g in range(0, nchunks, GW)]

                def emit_group(gi):
                    grp = groups[gi]
                    pm = psum_m.tile([128, 512], f32, tag="pm")
                    for jj, j in enumerate(grp):
                        co, rj = chks[j]
                        k = 0
                        for src, ws in nbr_srcs:
                            for w in ws:
                                nc.tensor.matmul(pm[:rj, jj * 128:jj * 128 + 128],
                                                 src[:, co:co + rj], w,
                                                 start=(k == 0),
                                                 stop=(k == nterm - 1))
                                k += 1
                    wd_g = len(grp) * 128
                    mhi = mp.tile([128, 512], bf16, tag="mhi")
                    if gi % 2 == 0:
                        nc.vector.tensor_copy(out=mhi[:, :wd_g], in_=pm[:, :wd_g])
                    else:
                        nc.scalar.copy(mhi[:, :wd_g], pm[:, :wd_g])
                    return mhi

                gq = [emit_group(0)]
                # self path: bf16 weights against bf16 state
                k = 0
                for src, ws in self_srcs:
                    for w in ws:
                        for a, off, wd in aggs:
                            nc.tensor.matmul(a[:, :wd], w, src[:, off:off + wd],
                                             start=(k == 0), stop=False)
                        k += 1
                for gi, grp in enumerate(groups):
                    mhi = gq.pop(0)
                    if gi + 1 < len(groups):
                        gq.append(emit_group(gi + 1))
                    for jj, j in enumerate(grp):
                        rj = chks[j][1]
                        base = moff + j * p_out
                        for a, off, wd in aggs:
                            nc.tensor.matmul(a[:, :wd],
                                             mhi[:rj, jj * 128:jj * 128 + 128],
                                             blob[:rj, base + off:base + off + wd],
                                             start=False,
                                             stop=(j == nchunks - 1))
                for a, off, wd in aggs:
                    nc.scalar.activation(out_tile[:, off:off + wd],
                                         a[:, :wd], Act.Relu,
                                         bias=bias_ap(bias_col))
                if col0_out is not None:
                    # un-rounded f32 copy of column 0 straight from PSUM
                    # (head input) — no extraction hop off the state tile
                    nc.scalar.activation(col0_out, aggs[0][0][:, 0:1],
                                         Act.Relu, bias=bias_ap(bias_col))

            def slot_stages(s):
                """Emission closures for one slot: [embed, base1, adapt1,
                base2, ...].  Two slots are interleaved stage-by-stage so
                each layer-boundary ACT wait is hidden under the other
                slot's matmuls."""
                Pm1, P0, P1, P2, P3 = slot_sizes[s]
                lay = lays[s]
                blob = blob_t[s]
                psz = [P0, P1, P2, P3, P4]
                xT = state.tile([128, Pm1], bf16, tag=f"x{s}", name=f"x{s}")
                lat = [xT] + [state.tile([128, psz[k]], bf16, tag=f"lat{k+1}_{s}",
                                         name=f"lat{k+1}_{s}")
                              for k in range(L)]
                currs = [xT] + [state.tile([128, psz[k + 1]], bf16,
                                           tag=f"curr{k+1}_{s}",
                                           name=f"curr{k+1}_{s}")
                                for k in range(L)]
                pins = [Pm1, P0, P1, P2]
                stages = []

                def embed_stage():
                    eoff = eoffs[s]
                    ept = ep_holder[0]
                    for i_sp, (a, off, wd) in enumerate(get_aggs(Pm1)):
                        nc.tensor.matmul(a[:, :wd], embw_hi[:40, :],
                                         ept[:, eoff + off:eoff + off + wd],
                                         start=True, stop=False)
                        nc.tensor.matmul(a[:, :wd], embw_lo[:40, :],
                                         ept[:, eoff + off:eoff + off + wd],
                                         start=False, stop=True)
                        if i_sp % 2 == 0:
                            nc.vector.tensor_copy(out=xT[:, off:off + wd],
                                                  in_=a[:, :wd])
                        else:
                            nc.scalar.copy(xT[:, off:off + wd], a[:, :wd])
                stages.append(embed_stage)

                def base_stage(i):
                    def run():
                        gconv(blob, lay[f"m{i+1}"][0],
                              nbr_srcs=[(lat[i], [bwn_t[i]])],
                              self_srcs=[(lat[i], [bws_t[i]])],
                              p_in=pins[i], p_out=psz[i],
                              bias_col=BCOL_BASE + i, out_tile=lat[i + 1],
                              col0_out=(gbT[:, s:s + 1] if i == L - 1
                                        else None))
                    return run

                def adapt_stage(i):
                    def run():
                        # the last adapter output is only read at column 0
                        # (the head input): write it straight into gaT and
                        # skip the extraction hop on the critical tail
                        out_t = currs[i + 1] if i < L - 1 else gaT[:, s:s + 1]
                        gconv(blob, lay[f"m{i+2}"][0],
                              nbr_srcs=[(lat[i + 1], [awn_t[i][0]]),
                                        (currs[i], [awn_t[i][1]])],
                              self_srcs=[(lat[i + 1], [aws_t[i][0]]),
                                         (currs[i], [aws_t[i][1]])],
                              p_in=psz[i], p_out=psz[i + 1],
                              bias_col=BCOL_ADAPT + i, out_tile=out_t)
                    return run

                # base_{i+1} ahead of adapt_i: they are independent, so in
                # the drained tail the base chain advances while the adapter
                # fills its ACT waits (critical depth ~6 instead of 8)
                stages.append(base_stage(0))
                for i in range(L - 1):
                    stages.append(base_stage(i + 1))
                    stages.append(adapt_stage(i))
                stages.append(adapt_stage(L - 1))
                return stages


            # ---- regression head (all slots at once) ----
            def whole_pass():
                load_blobs()
                streams = [slot_stages(ss) for ss in SKEW_ORDER]
                nst = len(streams[0])
                for r in range(nst + len(streams) - 1):
                    for i, stream in enumerate(streams):
                        k = r - i
                        if 0 <= k < nst:
                            stream[k]()
                emit_head()

            def head_mm(lhsT, rhs, bias_col, func):
                pm = psum_m.tile([128, 128], f32, tag="pm")
                nc.tensor.matmul(pm[:, :NG], lhsT, rhs, start=True, stop=True)
                out = state.tile([128, NG], f32, tag="hy")
                nc.scalar.activation(out[:], pm[:, :NG], func,
                                     bias=bias_ap(bias_col))
                return out

            def emit_head():
                # head with relu-free pairs constant-folded on host:
                # y = ((relu(g@Wa+ba))@Wb+bb -> relu) @ hW5 + hb5
                pm = psum_m.tile([128, 128], f32, tag="pm")
                nc.tensor.matmul(pm[:, :NG], hwa0, gbT[:],
                                 start=True, stop=False)
                nc.tensor.matmul(pm[:, :NG], hwa1, gaT[:],
                                 start=False, stop=True)
                y1 = state.tile([128, NG], f32, tag="hy")
                nc.scalar.activation(y1[:], pm[:, :NG], Act.Relu,
                                     bias=bias_ap(BCOL_HB1))
                y2 = head_mm(hwb, y1[:], BCOL_HMID + 0, Act.Relu)
                pm5 = psum_m.tile([128, 128], f32, tag="pm")
                nc.tensor.matmul(pm5[:1, :NG], hw5, y2[:],
                                 start=True, stop=True)
                yout = state.tile([1, NG], f32, tag="yout")
                nc.scalar.activation(yout[:], pm5[:1, :NG], Act.Identity,
                                     bias=bias_ap(BCOL_HB5)[:1])
                nc.sync.dma_start(y_d[:], yout[:])

            for _rep in range(reps):
                whole_pass()

    nc.compile()
    return nc


_NC_CACHE = {}
_LAST = {}


def _get_program(reps=1):
    key = (_LAST["slot_sizes"], reps)
    if key not in _NC_CACHE:
        _NC_CACHE[key] = _build_program(_LAST["slot_sizes"], reps=reps)
    return _NC_CACHE[key]


def _cones(edge, last_idx):
    """Nested cone ordering per graph.  Returns (order, sizes[n4..nm1])."""
    out = []
    for g in range(B):
        src, dst = edge[g, 0], edge[g, 1]
        order = [int(last_idx[g])]
        inset = np.zeros(N, bool)
        inset[order[0]] = True
        sizes = [1]
        for _ in range(5):
            new = np.unique(src[inset[dst]])
            new = new[~inset[new]]
            order.extend(new.tolist())
            inset[new] = True
            sizes.append(len(order))
        out.append((np.asarray(order), sizes))
    return out


def _split_hilo(a):
    hi = a.astype(BF16)
    lo = (a - hi.astype(F32)).astype(BF16)
    return hi, lo


def _prep_inputs(inputs):
    """Host-side cone construction + sharding.  Returns list of in_maps."""
    inds = np.asarray(inputs["regular_node_inds"]).astype(np.int64)
    shapes = np.asarray(inputs["regular_node_shapes"], dtype=F32)
    edge = np.asarray(inputs["edge_index"]).astype(np.int64)
    last_idx = np.asarray(inputs["last_idx"]).astype(np.int64)

    cones = _cones(edge, last_idx)
    # sort graphs by cost; slot j <- ranks [8j, 8j+8), core c <- rank 8j+c
    cost = np.array([c[1][5] + c[1][4] for c in cones])
    ranks = np.argsort(-cost, kind="stable")
    assign = ranks.reshape(NG, N_CORES)          # [slot, core] -> graph id
    slot_sizes = []
    for s in range(NG):
        gs = assign[s]
        mx = [max(cones[g][1][k] for g in gs) for k in range(6)]
        # sizes[k] = |V_{4-k}|; exact per-level maxes (Pm1,P0,P1,P2,P3)
        slot_sizes.append(tuple(int(mx[5 - l]) for l in range(5)))
    slot_sizes = tuple(slot_sizes)
    _LAST["slot_sizes"] = slot_sizes
    _LAST["assign"] = assign
    lays = [_blob_layout(s) for s in slot_sizes]

    # embed weights, hi/lo bf16 pair (exact): rows 0..31 table, 32..35 and
    # 36..39 shape_w (paired against shapes_hi / shapes_lo blob rows)
    embed_w = np.zeros((128, H), dtype=F32)
    embed_w[:VOCAB] = np.asarray(inputs["embed_table"], dtype=F32)
    embed_w[VOCAB:VOCAB + 4] = np.asarray(inputs["shape_w"], dtype=F32)
    embed_w[VOCAB + 4:VOCAB + 8] = np.asarray(inputs["shape_w"], dtype=F32)
    ehi, elo = _split_hilo(embed_w)
    # the shape_w rows must stay IDENTICAL in both copies within each of
    # hi/lo (they are), pairing: x = oh@(thi+tlo) + (shi+slo)@(swhi+swlo)
    embed_w2 = np.stack([ehi, elo], axis=1)     # [128, 2, H]

    bws2 = np.asarray(inputs["base_Wself"], dtype=F32).astype(BF16)
    bwn2 = np.asarray(inputs["base_Wnbr"], dtype=F32).astype(BF16)
    aws = np.asarray(inputs["adapt_Wself"], dtype=F32).reshape(L, 2, H, H)
    awn = np.asarray(inputs["adapt_Wnbr"], dtype=F32).reshape(L, 2, H, H)
    aws2 = np.ascontiguousarray(aws.transpose(0, 2, 1, 3)).astype(BF16)
    awn2 = np.ascontiguousarray(awn.transpose(0, 2, 1, 3)).astype(BF16)
    hW1 = np.asarray(inputs["hW1"], np.float64)
    hb1 = np.asarray(inputs["hb1"], np.float64)
    hWm = np.asarray(inputs["hWmid"], np.float64)
    hbm = np.asarray(inputs["hbmid"], np.float64)
    Wa = hW1 @ hWm[0]                       # [2H, H]
    ba = hb1 @ hWm[0] + hbm[0]
    Wb = hWm[1] @ hWm[2]                    # [H, H]
    bb = hbm[1] @ hWm[2] + hbm[2]
    hw1 = np.ascontiguousarray(
        Wa.astype(F32).reshape(2, H, H).transpose(1, 0, 2))

    biases = np.zeros((H, NBCOL), dtype=F32)
    biases[:, BCOL_BASE:BCOL_BASE + L] = np.asarray(inputs["base_b"], dtype=F32).T
    biases[:, BCOL_ADAPT:BCOL_ADAPT + L] = np.asarray(inputs["adapt_b"], dtype=F32).T
    biases[:, BCOL_HB1] = ba.astype(F32)
    biases[:, BCOL_HMID] = bb.astype(F32)
    biases[0, BCOL_HB5] = np.asarray(inputs["hb5"], dtype=F32)[0]

    WB = 2 * H + L * 6 * H
    WF = 3 * H + 1 + NBCOL
    wpack_bf = np.zeros((128, WB), dtype=BF16)
    wpack_bf[:, 0:H] = embed_w2[:, 0, :]
    wpack_bf[:, H:2 * H] = embed_w2[:, 1, :]
    for i in range(L):
        o = 2 * H + i * 6 * H
        wpack_bf[:, o:o + H] = bwn2[i]
        wpack_bf[:, o + H:o + 2 * H] = bws2[i]
        wpack_bf[:, o + 2 * H:o + 3 * H] = awn2[i][:, 0, :]
        wpack_bf[:, o + 3 * H:o + 4 * H] = awn2[i][:, 1, :]
        wpack_bf[:, o + 4 * H:o + 5 * H] = aws2[i][:, 0, :]
        wpack_bf[:, o + 5 * H:o + 6 * H] = aws2[i][:, 1, :]
    wpack_f32 = np.zeros((128, WF), dtype=F32)
    wpack_f32[:, 0:H] = Wa.astype(F32)[:H, :]
    wpack_f32[:, H:2 * H] = Wa.astype(F32)[H:, :]
    wpack_f32[:, 2 * H:3 * H] = Wb.astype(F32)
    wpack_f32[:, 3 * H:3 * H + 1] = np.asarray(inputs["hW5"], dtype=F32)
    wpack_f32[:, 3 * H + 1:] = biases
    shared = {"wpack_bf": wpack_bf, "wpack_f32": wpack_f32}
    in_maps = [dict(shared) for _ in range(N_CORES)]
    EP = sum(sz[0] for sz in slot_sizes)
    epack = [np.zeros((40, EP), dtype=BF16) for _ in range(N_CORES)]
    for s in range(NG):
        Pm1, P0, P1, P2, P3 = slot_sizes[s]
        lay = lays[s]
        for c in range(N_CORES):
            g = assign[s, c]
            order, sizes = cones[g]
            n = len(order)
            pos = np.full(N, -1, np.int64)
            pos[order] = np.arange(n)
            src, dst = edge[g, 0], edge[g, 1]
            ps, pd = pos[src], pos[dst]
            blob = np.zeros((128, lay["_total"]), dtype=FP8)
            # erhs: one-hot rows 0..31, shapes hi rows 32..35, lo rows 36..39
            eoff = sum(sz[0] for sz in slot_sizes[:s])
            erhs = np.zeros((40, Pm1), dtype=F32)
            erhs[inds[g][order], np.arange(n)] = 1.0
            shi, slo = _split_hilo(shapes[g][order].T)
            epack[c][:, eoff:eoff + Pm1] = erhs.astype(BF16)
            epack[c][VOCAB:VOCAB + 4, eoff:eoff + n] = shi[:, :n]
            epack[c][VOCAB + 4:VOCAB + 8, eoff:eoff + n] = slo[:, :n]
            for l, (pin, pout) in enumerate(lay["_dims"]):
                ncols = sizes[4 - l]   # |V_{l-1}|
                rceil = ((pin + 127) // 128) * 128
                M = np.zeros((rceil, pout), dtype=F32)
                mask = (pd >= 0) & (pd < ncols)
                np.add.at(M, (ps[mask], pd[mask]), 1.0)
                moff = lay[f"m{l + 1}"][0]
                # chunk-major on the free axis, stride pout, exact widths
                assert M.max() <= 16, "edge multiplicity exceeds fp8-exact range"
                Mt = M.astype(FP8).reshape(rceil // 128, 128, pout)
                blob[:, moff:moff + (rceil // 128) * pout] = (
                    Mt.transpose(1, 0, 2).reshape(128, -1))
            in_maps[c][f"blob{s}"] = blob
    for c in range(N_CORES):
        in_maps[c]["epack"] = epack[c]
    return in_maps


def kernel(**inputs) -> np.ndarray:
    in_maps = _prep_inputs(inputs)
    nc = _get_program()
    assign = _LAST["assign"]
    # first dispatch after a fresh compile has produced garbage before
    # (axon staging race); run twice and keep the steady-state result
    run_bass_kernel_spmd(nc, in_maps, core_ids=list(range(N_CORES)))
    res = run_bass_kernel_spmd(nc, in_maps, core_ids=list(range(N_CORES)))
    out = np.zeros((B, 1), dtype=F32)
    for c in range(N_CORES):
        yc = np.asarray(res.results[c]["y"]).reshape(NG)
        for s in range(NG):
            out[assign[s, c], 0] = yc[s]
    return out
